# revision 10
# baseline (speedup 1.0000x reference)
"""Multi-head attention (B=2, S=2048, D=1024, H=16) on 8 Trainium2 cores.

Sharding: core c handles batch b=c//4 and head group g=c%4 (4 heads,
d_model slice of 256). Q/K/V/O projections are tensor-parallel over the
head dimension; attention is fully local per core; the output projection
produces per-core partial sums over d_model that the host reduces
(row-parallel W_o, "all-reduce" done in the unshard step).

Returns (output [B,S,D] fp32, attn_weights [B,H,S,S] fp32), matching the
reference module.
"""

import os
import numpy as np

import concourse.bass as bass
import concourse.bacc as bacc
import concourse.mybir as mybir
import concourse.tile as tile
from concourse.bass_utils import run_bass_kernel_spmd

F32 = mybir.dt.float32
F32R = mybir.dt.float32r
AF = mybir.ActivationFunctionType
AX = mybir.AxisListType

B, S, D, H = 2, 2048, 1024, 16
HG = 4              # heads per core
DC = 256            # d_model slice per core (HG * 64)
HD = 64             # head dim
NCORES = 8
SCALE = 0.125       # 1/sqrt(head_dim)
NEG = -1.0e30

QC = 128            # natural-layout sq chunk (psum partitions)
KBLK = 512          # natural-layout sk block (psum free dim)
KT = 128            # transposed-layout sk tile (psum partitions)
JB = 512            # transposed-layout sq block (psum free dim)

NQC, NKB = S // QC, S // KBLK     # 16, 4
NKT, NJB = S // KT, S // JB       # 16, 4

SKIP, FULL, MIXED = 0, 1, 2

LAST_EXEC_NS = None
LAST_RESULTS = None

_prog_cache = {}


def _classify(mask):
    """Per-tile classification over BOTH batches (union-active /
    intersection-full), for the natural [QC x KBLK] and transposed
    [KT x JB] grids. mask: [B, S, S] bool."""
    m = mask.reshape(B, NQC, QC, NKB, KBLK)
    any_n = m.any(axis=(0, 2, 4))          # [NQC, NKB]
    all_n = m.all(axis=(0, 2, 4))
    nat = np.where(all_n, FULL, np.where(any_n, MIXED, SKIP)).astype(np.int32)

    mt = mask.swapaxes(1, 2).reshape(B, NKT, KT, NJB, JB)
    any_t = mt.any(axis=(0, 2, 4))         # [NKT, NJB]
    all_t = mt.all(axis=(0, 2, 4))
    tct = np.where(all_t, FULL, np.where(any_t, MIXED, SKIP)).astype(np.int32)
    return nat, tct


def _build(nat_cls, tct_cls, n_bias_n, n_bias_t, bias_resident):
    """Build + compile the per-core program. All cores run the same
    program; per-core data differences come through the input tensors."""
    nc = bacc.Bacc("TRN2", target_bir_lowering=False, debug=False,
                   num_devices=NCORES)

    qryT_d = nc.dram_tensor("qryT", [D, S], F32R, kind="ExternalInput")
    keyT_d = nc.dram_tensor("keyT", [D, S], F32R, kind="ExternalInput")
    valT_d = nc.dram_tensor("valT", [D, S], F32R, kind="ExternalInput")
    wqT_d = nc.dram_tensor("wqT", [D, DC], F32R, kind="ExternalInput")
    wkT_d = nc.dram_tensor("wkT", [D, DC], F32R, kind="ExternalInput")
    wvT_d = nc.dram_tensor("wvT", [D, DC], F32R, kind="ExternalInput")
    woT_d = nc.dram_tensor("woT", [DC, D], F32R, kind="ExternalInput")
    bn_d = bt_d = None
    if n_bias_n:
        bn_d = nc.dram_tensor("biasN", [n_bias_n, QC, KBLK], F32,
                              kind="ExternalInput")
    if n_bias_t:
        bt_d = nc.dram_tensor("biasT", [n_bias_t, KT, JB], F32,
                              kind="ExternalInput")

    attn_d = nc.dram_tensor("attn", [HG, S, S], F32, kind="ExternalOutput")
    outp_d = nc.dram_tensor("outp", [S, D], F32, kind="ExternalOutput")
    denr_d = nc.dram_tensor("denr", [HG * NJB, JB], F32)  # internal scratch

    r = F32R

    with tile.TileContext(nc) as tc:
        with (
            tc.tile_pool(name="res", bufs=1) as res,
            tc.tile_pool(name="inq", bufs=3) as inq,
            tc.tile_pool(name="inv", bufs=4) as inv,
            tc.tile_pool(name="ext", bufs=4) as extp,
            tc.tile_pool(name="arow", bufs=2) as arow,
            tc.tile_pool(name="btl", bufs=2) as btl,
            tc.tile_pool(name="sml", bufs=2) as sml,
            tc.tile_pool(name="osb", bufs=2) as osb,
            tc.tile_pool(name="ps", bufs=2, space="PSUM") as ps,
        ):
            # ---- resident SBUF tensors ----
            qT = res.tile([128, 2, S], F32R)     # q^T: d-slice rows, s cols
            kT = res.tile([128, 2, S], F32R)
            fT = res.tile([128, 2, S], F32R)     # normalized feats^T
            v1s = res.tile([128, NKT, HG, HD + 1], F32R)  # [v | ones] per sk-chunk
            wq = res.tile([128, 8, DC], F32R)
            wk = res.tile([128, 8, DC], F32R)
            wv = res.tile([128, 8, DC], F32R)
            wo = res.tile([128, 2, D], F32R)
            zrow = res.tile([128, S], F32)
            if bias_resident:
                bnres = res.tile([128, max(n_bias_n, 1), KBLK], F32)
                btres = res.tile([128, max(n_bias_t, 1), JB], F32)

            nc.vector.memset(zrow[:], 0.0)
            # fp32r tiles cannot be memset directly; write the denominator
            # ones-column via a rounding copy from an fp32 ones tile
            ones_c = res.tile([128, NKT * HG], F32)
            nc.vector.memset(ones_c[:], 1.0)
            nc.vector.tensor_copy(
                v1s[:, :, :, HD],
                ones_c[:].rearrange("p (a b) -> p a b", b=HG))

            nc.sync.dma_start(wq[:], wqT_d.ap().rearrange("(c p) n -> p c n", p=128))
            nc.sync.dma_start(wk[:], wkT_d.ap().rearrange("(c p) n -> p c n", p=128))
            nc.sync.dma_start(wv[:], wvT_d.ap().rearrange("(c p) n -> p c n", p=128))
            nc.sync.dma_start(wo[:], woT_d.ap().rearrange("(c p) n -> p c n", p=128))
            if bias_resident:
                if n_bias_n:
                    nc.sync.dma_start(
                        bnres[:, 0:n_bias_n, :],
                        bn_d.ap().rearrange("t p n -> p t n"))
                if n_bias_t:
                    nc.sync.dma_start(
                        btres[:, 0:n_bias_t, :],
                        bt_d.ap().rearrange("t p n -> p t n"))

            # ---- Phase A: projections ----
            # q^T / k^T: [d_c, S] = (W.T slice).T @ x^T, accumulated over D
            for (w_s, x_d, dst) in ((wq, qryT_d, qT), (wk, keyT_d, kT)):
                for sb in range(S // 512):
                    psums = []
                    for hp in range(2):
                        p = ps.tile([128, 512], F32, name=f"pp{hp}", tag="mm")
                        psums.append(p)
                    for dc in range(8):
                        xin = inq.tile([128, 512], F32R, name="xin")
                        nc.sync.dma_start(
                            xin[:],
                            x_d.ap()[dc * 128:(dc + 1) * 128,
                                     sb * 512:(sb + 1) * 512])
                        for hp in range(2):
                            nc.tensor.matmul(
                                psums[hp][:],
                                w_s[:, dc, hp * 128:(hp + 1) * 128],
                                xin[:],
                                start=(dc == 0), stop=(dc == 7))
                    for hp in range(2):
                        nc.vector.tensor_copy(
                            dst[:, hp, sb * 512:(sb + 1) * 512], psums[hp][:])

            # v natural layout: [s, d_c] = (val^T tile).T @ (W_v.T slice)
            for sc in range(NKT):
                pv = ps.tile([128, DC], F32, name="pv", tag="mm")
                for dc in range(8):
                    vin = inv.tile([128, 128], F32R, name="vin")
                    nc.gpsimd.dma_start(
                        vin[:],
                        valT_d.ap()[dc * 128:(dc + 1) * 128,
                                    sc * 128:(sc + 1) * 128])
                    nc.tensor.matmul(pv[:], vin[:],
                                     wv[:, dc, :],
                                     start=(dc == 0), stop=(dc == 7))
                for h in range(HG):
                    nc.vector.tensor_copy(v1s[:, sc, h, 0:HD],
                                          pv[:, h * HD:(h + 1) * HD])

            # ---- Phase B: transposed scores -> exp -> feats^T (+denom) ----
            for h in range(HG):
                hp, poff = h // 2, (h % 2) * HD
                for j in range(NJB):
                    acts = [kt for kt in range(NKT) if tct_cls[kt, j] != SKIP]
                    fsl = fT[poff:poff + HD, hp, j * JB:(j + 1) * JB]
                    if not acts:
                        nc.vector.memset(fsl, 0.0)
                        continue
                    pf = ps.tile([HD + 1, JB], F32, name="pf", tag="ft")
                    for i, kt in enumerate(acts):
                        pst = ps.tile([128, JB], F32, name="pst", tag="mm")
                        nc.tensor.matmul(
                            pst[:],
                            kT[poff:poff + HD, hp,
                               kt * KT:(kt + 1) * KT],
                            qT[poff:poff + HD, hp,
                               j * JB:(j + 1) * JB],
                            start=True, stop=True)
                        if tct_cls[kt, j] == MIXED:
                            bi = bias_idx_t[(kt, j)]
                            if bias_resident:
                                bsrc = btres[:, bi, :]
                            else:
                                bb = btl.tile([128, JB], F32, name="btt")
                                nc.gpsimd.dma_start(bb[:], bt_d.ap()[bi])
                                bsrc = bb[:]
                            nc.vector.tensor_add(pst[:], pst[:], bsrc)
                        ext = extp.tile([128, JB], F32R, name="ext")
                        nc.scalar.activation(ext[:], pst[:], AF.Exp, scale=SCALE)
                        nc.tensor.matmul(
                            pf[:], v1s[:, kt, h, :],
                            ext[:],
                            start=(i == 0), stop=(i == len(acts) - 1))
                    # denominator row -> reciprocal -> broadcast via DRAM
                    rr = sml.tile([1, JB], F32, name="rr")
                    nc.vector.reciprocal(rr[:], pf[HD:HD + 1, :])
                    scr = denr_d.ap()[h * NJB + j:h * NJB + j + 1, :]
                    nc.gpsimd.dma_start(scr, rr[:])
                    rbc = sml.tile([HD, JB], F32, name="rbc")
                    nc.gpsimd.dma_start(
                        rbc[:],
                        bass.AP(tensor=scr.tensor, offset=scr.offset,
                                ap=[[0, HD]] + scr.ap[1:]))
                    nc.vector.tensor_mul(fsl, pf[0:HD, :], rbc[:])

            # ---- Phase C: natural scores -> exp(+accum) -> normalize -> DMA
            for h in range(HG):
                hp, poff = h // 2, (h % 2) * HD
                for qc in range(NQC):
                    acts = [kb for kb in range(NKB) if nat_cls[qc, kb] != SKIP]
                    if not acts:
                        nc.sync.dma_start(
                            attn_d.ap()[h, qc * QC:(qc + 1) * QC, :],
                            zrow[:, :])
                        continue
                    nend = max(acts) + 1
                    ar = arow.tile([128, S], F32, name="ar")
                    dp = sml.tile([128, NKB], F32, name="dp")
                    # pair consecutive blocks into 2-bank psum tiles
                    kb = 0
                    npart = 0
                    while kb < nend:
                        take = 2 if kb + 1 < nend else 1
                        pn = ps.tile([128, 1024], F32, name="pn", tag="nat")
                        width = 0
                        for t in range(take):
                            blk = kb + t
                            if nat_cls[qc, blk] == SKIP:
                                nc.vector.memset(
                                    ar[:, blk * KBLK:(blk + 1) * KBLK], 0.0)
                                continue
                            psl = pn[:, t * KBLK:(t + 1) * KBLK]
                            nc.tensor.matmul(
                                psl,
                                qT[poff:poff + HD, hp,
                                   qc * QC:(qc + 1) * QC],
                                kT[poff:poff + HD, hp,
                                   blk * KBLK:(blk + 1) * KBLK],
                                start=True, stop=True)
                            if nat_cls[qc, blk] == MIXED:
                                bi = bias_idx_n[(qc, blk)]
                                if bias_resident:
                                    bsrc = bnres[:, bi, :]
                                else:
                                    bb = btl.tile([128, KBLK], F32, name="btn")
                                    nc.gpsimd.dma_start(bb[:], bn_d.ap()[bi])
                                    bsrc = bb[:]
                                nc.vector.tensor_add(psl, psl, bsrc)
                            width += 1
                        if width == 2:
                            nc.scalar.activation(
                                ar[:, kb * KBLK:(kb + 2) * KBLK], pn[:],
                                AF.Exp, scale=SCALE,
                                accum_out=dp[:, npart:npart + 1])
                            npart += 1
                        elif width == 1:
                            blk = kb if nat_cls[qc, kb] != SKIP else kb + 1
                            nc.scalar.activation(
                                ar[:, blk * KBLK:(blk + 1) * KBLK],
                                pn[:, (blk - kb) * KBLK:(blk - kb + 1) * KBLK],
                                AF.Exp, scale=SCALE,
                                accum_out=dp[:, npart:npart + 1])
                            npart += 1
                        kb += take
                    dsum = sml.tile([128, 1], F32, name="dsum")
                    if npart > 1:
                        nc.vector.reduce_sum(dsum[:], dp[:, 0:npart], axis=AX.X)
                    else:
                        nc.vector.tensor_copy(dsum[:], dp[:, 0:1])
                    nc.vector.reciprocal(dsum[:], dsum[:])
                    nc.vector.tensor_scalar_mul(
                        ar[:, 0:nend * KBLK], ar[:, 0:nend * KBLK], dsum[:])
                    nc.sync.dma_start(
                        attn_d.ap()[h, qc * QC:(qc + 1) * QC, 0:nend * KBLK],
                        ar[:, 0:nend * KBLK])
                    if nend < NKB:
                        nc.sync.dma_start(
                            attn_d.ap()[h, qc * QC:(qc + 1) * QC,
                                        nend * KBLK:S],
                            zrow[:, 0:(NKB - nend) * KBLK])

            # ---- Phase D: partial output projection ----
            for qc in range(NQC):
                ot = osb.tile([128, D], F32, name="ot")
                for nh in range(2):
                    po = ps.tile([128, 512], F32, name="po", tag="mm")
                    for d2 in range(2):
                        nc.tensor.matmul(
                            po[:],
                            fT[:, d2, qc * QC:(qc + 1) * QC],
                            wo[:, d2, nh * 512:(nh + 1) * 512],
                            start=(d2 == 0), stop=(d2 == 1))
                    nc.vector.tensor_copy(ot[:, nh * 512:(nh + 1) * 512], po[:])
                nc.sync.dma_start(
                    outp_d.ap()[qc * QC:(qc + 1) * QC, :], ot[:])

    nc.compile()
    return nc


def kernel(qry, key, val, attn_mask, Wq, Wk, Wv, Wo):
    global LAST_EXEC_NS, LAST_RESULTS, bias_idx_n, bias_idx_t

    qry = np.asarray(qry, np.float32)
    key = np.asarray(key, np.float32)
    val = np.asarray(val, np.float32)
    mask = np.asarray(attn_mask).astype(bool)
    Wq, Wk, Wv, Wo = (np.asarray(w, np.float32) for w in (Wq, Wk, Wv, Wo))

    nat_cls, tct_cls = _classify(mask)

    # bias tiles for mixed tiles (shared across batches via the union
    # classification; per-core data still uses the core's own batch mask).
    # Deduped by content (consistent across both batches): a causal mask
    # has only 4 distinct diagonal-crossing patterns per layout.
    biasf = np.where(mask, np.float32(0), np.float32(NEG))  # [B, S, S]
    biasfT = np.ascontiguousarray(biasf.swapaxes(1, 2))

    bias_idx_n, bias_idx_t = {}, {}
    rep_n, rep_t = [], []  # representative (qc,kb)/(kt,j) per unique index
    uniq_n, uniq_t = {}, {}
    for qc in range(NQC):
        for kb in range(NKB):
            if nat_cls[qc, kb] == MIXED:
                t0 = biasf[0, qc * QC:(qc + 1) * QC, kb * KBLK:(kb + 1) * KBLK]
                t1 = biasf[1, qc * QC:(qc + 1) * QC, kb * KBLK:(kb + 1) * KBLK]
                k = (t0.tobytes(), t1.tobytes())
                if k not in uniq_n:
                    uniq_n[k] = len(uniq_n)
                    rep_n.append((qc, kb))
                bias_idx_n[(qc, kb)] = uniq_n[k]
    for kt in range(NKT):
        for j in range(NJB):
            if tct_cls[kt, j] == MIXED:
                t0 = biasfT[0, kt * KT:(kt + 1) * KT, j * JB:(j + 1) * JB]
                t1 = biasfT[1, kt * KT:(kt + 1) * KT, j * JB:(j + 1) * JB]
                k = (t0.tobytes(), t1.tobytes())
                if k not in uniq_t:
                    uniq_t[k] = len(uniq_t)
                    rep_t.append((kt, j))
                bias_idx_t[(kt, j)] = uniq_t[k]
    n_bn, n_bt = len(rep_n), len(rep_t)
    bias_resident = (n_bn + n_bt) <= 16

    cache_key = (nat_cls.tobytes(), tct_cls.tobytes(), bias_resident)
    if cache_key not in _prog_cache:
        _prog_cache[cache_key] = _build(nat_cls, tct_cls, n_bn, n_bt,
                                        bias_resident)
    nc = _prog_cache[cache_key]

    # per-batch host prep
    in_maps = []
    for c in range(NCORES):
        b, g = c // 4, c % 4
        cs = slice(DC * g, DC * g + DC)
        m = {
            "qryT": np.ascontiguousarray(qry[b].T),
            "keyT": np.ascontiguousarray(key[b].T),
            "valT": np.ascontiguousarray(val[b].T),
            "wqT": np.ascontiguousarray(Wq.T[:, cs]),
            "wkT": np.ascontiguousarray(Wk.T[:, cs]),
            "wvT": np.ascontiguousarray(Wv.T[:, cs]),
            "woT": np.ascontiguousarray(Wo.T[cs, :]),
        }
        if n_bn:
            bn = np.empty((n_bn, QC, KBLK), np.float32)
            for i, (qc, kb) in enumerate(rep_n):
                bn[i] = biasf[b, qc * QC:(qc + 1) * QC,
                              kb * KBLK:(kb + 1) * KBLK]
            m["biasN"] = bn
        if n_bt:
            bt = np.empty((n_bt, KT, JB), np.float32)
            for i, (kt, j) in enumerate(rep_t):
                bt[i] = biasfT[b, kt * KT:(kt + 1) * KT, j * JB:(j + 1) * JB]
            m["biasT"] = bt
        in_maps.append(m)

    res = run_bass_kernel_spmd(nc, in_maps, list(range(NCORES)))
    LAST_EXEC_NS = res.exec_time_ns
    LAST_RESULTS = res
    globals()["last_nc"] = nc
    globals()["last_in_maps"] = in_maps

    attn_weights = np.empty((B, H, S, S), np.float32)
    output64 = np.zeros((B, S, D), np.float64)
    for c in range(NCORES):
        b, g = c // 4, c % 4
        attn_weights[b, HG * g:HG * g + HG] = res.results[c]["attn"]
        output64[b] += res.results[c]["outp"]
    output = output64.astype(np.float32)
    return output, attn_weights


# revision 34
# speedup vs baseline: 318.6744x; 318.6744x over previous
"""Multi-head attention (B=2, S=2048, D=1024, H=16) on 8 Trainium2 cores.

Sharding: core c handles batch b=c//4 and head group g=c%4 (4 heads,
d_model slice of 256). Q/K/V/O projections are tensor-parallel over the
head dimension; attention is fully local per core; the output projection
produces per-core partial sums over d_model that the host reduces
(row-parallel W_o, "all-reduce" done in the unshard step).

Returns (output [B,S,D] fp32, attn_weights [B,H,S,S] fp32), matching the
reference module.
"""

import os
import numpy as np

import concourse.bass as bass
import concourse.bacc as bacc
import concourse.mybir as mybir
import concourse.tile as tile
from concourse.bass_utils import run_bass_kernel_spmd

F32 = mybir.dt.float32
F32R = mybir.dt.float32r
AF = mybir.ActivationFunctionType
AX = mybir.AxisListType

B, S, D, H = 2, 2048, 1024, 16
HG = 4              # heads per core
DC = 256            # d_model slice per core (HG * 64)
HD = 64             # head dim
NCORES = 8
SCALE = 0.125       # 1/sqrt(head_dim)
NEG = -1.0e30

QC = 128            # natural-layout sq chunk (psum partitions)
KBLK = 512          # natural-layout sk block (psum free dim)
KT = 128            # transposed-layout sk tile (psum partitions)
JB = 512            # transposed-layout sq block (psum free dim)

NQC, NKB = S // QC, S // KBLK     # 16, 4
NKT, NJB = S // KT, S // JB       # 16, 4

SKIP, FULL, MIXED = 0, 1, 2

LAST_EXEC_NS = None
LAST_RESULTS = None

_prog_cache = {}


def _classify(mask):
    """Per-tile classification over BOTH batches (union-active /
    intersection-full), for the natural [QC x KBLK] and transposed
    [KT x JB] grids. mask: [B, S, S] bool."""
    m = mask.reshape(B, NQC, QC, NKB, KBLK)
    any_n = m.any(axis=(0, 2, 4))          # [NQC, NKB]
    all_n = m.all(axis=(0, 2, 4))
    nat = np.where(all_n, FULL, np.where(any_n, MIXED, SKIP)).astype(np.int32)

    mt = mask.swapaxes(1, 2).reshape(B, NKT, KT, NJB, JB)
    any_t = mt.any(axis=(0, 2, 4))         # [NKT, NJB]
    all_t = mt.all(axis=(0, 2, 4))
    tct = np.where(all_t, FULL, np.where(any_t, MIXED, SKIP)).astype(np.int32)
    return nat, tct


def _build(nat_cls, tct_cls, n_bias_n, n_bias_t, bias_resident, ext_n, rng_t):
    """Build + compile the per-core program. All cores run the same
    program; per-core data differences come through the input tensors."""
    nc = bacc.Bacc("TRN2", target_bir_lowering=False, debug=False,
                   num_devices=NCORES)

    qryT_d = nc.dram_tensor("qryT", [D, S], F32R, kind="ExternalInput")
    keyT_d = nc.dram_tensor("keyT", [D, S], F32R, kind="ExternalInput")
    valT_d = nc.dram_tensor("valT", [D, S], F32R, kind="ExternalInput")
    wqT_d = nc.dram_tensor("wqT", [D, DC], F32R, kind="ExternalInput")
    wkT_d = nc.dram_tensor("wkT", [D, DC], F32R, kind="ExternalInput")
    wvT_d = nc.dram_tensor("wvT", [D, DC], F32R, kind="ExternalInput")
    woT_d = nc.dram_tensor("woT", [DC, D], F32R, kind="ExternalInput")
    bn_d = bt_d = None
    if n_bias_n:
        bn_d = nc.dram_tensor("biasN", [n_bias_n, QC, KBLK], F32R,
                              kind="ExternalInput")
    if n_bias_t:
        bt_d = nc.dram_tensor("biasT", [n_bias_t, KT, JB], F32R,
                              kind="ExternalInput")
    ident_d = nc.dram_tensor("ident", [128, 128], F32R, kind="ExternalInput")

    attn_d = nc.dram_tensor("attn", [HG, S, S], F32, kind="ExternalOutput")
    outp_d = nc.dram_tensor("outp", [S, D], F32, kind="ExternalOutput")
    denr_d = nc.dram_tensor("denr", [HG * NJB, JB], F32)  # internal scratch

    r = F32R

    with tile.TileContext(nc) as tc:
        with (
            tc.tile_pool(name="res", bufs=1) as res,
            tc.tile_pool(name="inq", bufs=3) as inq,
            tc.tile_pool(name="inv", bufs=9) as inv,
            tc.tile_pool(name="ext", bufs=6) as extp,
            tc.tile_pool(name="arow", bufs=5) as arow,
            tc.tile_pool(name="btl", bufs=2) as btl,
            tc.tile_pool(name="sml", bufs=2) as sml,
            tc.tile_pool(name="osb", bufs=2) as osb,
            tc.tile_pool(name="ps", bufs=2, space="PSUM") as ps,
        ):
            # ---- resident SBUF tensors ----
            qT = res.tile([128, 2, S], F32R)     # q^T: d-slice rows, s cols
            kT = res.tile([128, 2, S], F32R)
            fT = res.tile([128, 2, S], F32R)     # normalized feats^T
            v1s = res.tile([128, NKT, HG, HD + 1], F32R)  # [v | ones] per sk-chunk
            wq = res.tile([128, 8, DC], F32R)
            wk = res.tile([128, 8, DC], F32R)
            wv = res.tile([128, 8, DC], F32R)
            wo = res.tile([128, 2, D], F32R)
            ident = res.tile([128, 128], F32R)
            nc.sync.dma_start(ident[:], ident_d.ap())
            if bias_resident:
                bnres = res.tile([128, max(n_bias_n, 1), KBLK], F32R)
                btres = res.tile([128, max(n_bias_t, 1), JB], F32R)
            # fp32r tiles cannot be memset directly; write the denominator
            # ones-column via a rounding copy from an fp32 ones tile
            ones_c = res.tile([128, NKT * HG], F32)
            nc.vector.memset(ones_c[:], 1.0)
            nc.vector.tensor_copy(
                v1s[:, :, :, HD],
                ones_c[:].rearrange("p (a b) -> p a b", b=HG))

            nc.sync.dma_start(wq[:], wqT_d.ap().rearrange("(c p) n -> p c n", p=128))
            nc.sync.dma_start(wk[:], wkT_d.ap().rearrange("(c p) n -> p c n", p=128))
            nc.sync.dma_start(wv[:], wvT_d.ap().rearrange("(c p) n -> p c n", p=128))
            nc.sync.dma_start(wo[:], woT_d.ap().rearrange("(c p) n -> p c n", p=128))
            if bias_resident:
                if n_bias_n:
                    nc.sync.dma_start(
                        bnres[:, 0:n_bias_n, :],
                        bn_d.ap().rearrange("t p n -> p t n"))
                if n_bias_t:
                    nc.sync.dma_start(
                        btres[:, 0:n_bias_t, :],
                        bt_d.ap().rearrange("t p n -> p t n"))

            # ---- Phase A: projections, interleaved by s-block so phase B
            # can start after the first block ----
            for sb in range(S // 512):
                # q^T / k^T: [d_c, S] = (W.T slice).T @ x^T, over D chunks
                for (w_s, x_d, dst, tag) in ((wq, qryT_d, qT, "mm"),
                                             (wk, keyT_d, kT, "nat")):
                    psums = []
                    for hp in range(2):
                        p = ps.tile([128, 512], F32, name=f"pp{hp}", tag=tag)
                        psums.append(p)
                    for dc in range(8):
                        xin = inq.tile([128, 512], F32R, name="xin")
                        nc.sync.dma_start(
                            xin[:],
                            x_d.ap()[dc * 128:(dc + 1) * 128,
                                     sb * 512:(sb + 1) * 512])
                        for hp in range(2):
                            nc.tensor.matmul(
                                psums[hp][:],
                                w_s[:, dc, hp * 128:(hp + 1) * 128],
                                xin[:],
                                start=(dc == 0), stop=(dc == 7))
                    for hp in range(2):
                        nc.vector.tensor_copy(
                            dst[:, hp, sb * 512:(sb + 1) * 512], psums[hp][:])

                # v natural layout: [s, d_c] = (val^T tile).T @ (W_v.T slice)
                vins = []
                for dc in range(8):
                    vin = inv.tile([128, 512], F32R, name="vin")
                    nc.sync.dma_start(
                        vin[:],
                        valT_d.ap()[dc * 128:(dc + 1) * 128,
                                    sb * 512:(sb + 1) * 512])
                    vins.append(vin)
                for s4 in range(4):
                    sc = sb * 4 + s4
                    pv = ps.tile([128, DC], F32, name="pv", tag="ft")
                    for dc in range(8):
                        nc.tensor.matmul(
                            pv[:], vins[dc][:, s4 * 128:(s4 + 1) * 128],
                            wv[:, dc, :],
                            start=(dc == 0), stop=(dc == 7))
                    nc.vector.tensor_copy(
                        v1s[:, sc, :, 0:HD],
                        pv[:].rearrange("p (h d) -> p h d", h=HG))

            # ---- Phases B/C/D interleaved per sq-block j ----
            # B: transposed scores -> exp -> feats^T (+denom)
            # C: natural scores -> exp(+accum) -> normalize -> attn DMA
            # D: partial output projection (needs all heads' feats for j)
            def phase_b(h, j):
                    hp, poff = h // 2, (h % 2) * HD
                    acts = [kt for kt in range(NKT) if tct_cls[kt, j] != SKIP]
                    fsl = fT[poff:poff + HD, hp, j * JB:(j + 1) * JB]
                    if not acts:
                        nc.vector.memset(fsl, 0.0)
                        return
                    pf = ps.tile([HD + 1, JB], F32, name="pf", tag="ft")
                    qsl = qT[poff:poff + HD, hp, j * JB:(j + 1) * JB]
                    nacts = len(acts)
                    for i, kt in enumerate(acts):
                        mixed = tct_cls[kt, j] == MIXED
                        c0, c1 = 0, JB
                        if mixed and i > 0:
                            c0, c1 = rng_t[(kt, j)]
                        w = c1 - c0
                        pst = ps.tile([128, JB], F32, name="pst", tag="mm")
                        nc.tensor.matmul(
                            pst[:, c0:c1],
                            kT[poff:poff + HD, hp,
                               kt * KT:(kt + 1) * KT],
                            qsl[:, c0:c1], start=True, stop=not mixed)
                        if mixed:
                            bi = bias_idx_t[(kt, j)]
                            if bias_resident:
                                nc.tensor.matmul(pst[:, c0:c1], ident[:],
                                                 btres[:, bi, c0:c1],
                                                 start=False, stop=True)
                            else:
                                bb = btl.tile([128, JB], F32R, name="btt")
                                nc.gpsimd.dma_start(bb[:], bt_d.ap()[bi])
                                nc.tensor.matmul(pst[:, c0:c1], ident[:],
                                                 bb[:, c0:c1],
                                                 start=False, stop=True)
                        ext = extp.tile([128, JB], F32R, name="ext")
                        nc.scalar.activation(ext[:, c0:c1], pst[:, c0:c1],
                                             AF.Exp, scale=SCALE)
                        nc.tensor.matmul(
                            pf[:, c0:c1], v1s[:, kt, h, :], ext[:, c0:c1],
                            start=(i == 0), stop=(i == nacts - 1))
                    # denominator row -> reciprocal -> PE ones-broadcast
                    rr = sml.tile([1, JB], F32, name="rr")
                    nc.vector.reciprocal(rr[:], pf[HD:HD + 1, :])
                    scr = denr_d.ap()[h * NJB + j:h * NJB + j + 1, :]
                    nc.gpsimd.dma_start(scr, rr[:])
                    rbc = sml.tile([HD, JB], F32, name="rbc")
                    nc.gpsimd.dma_start(
                        rbc[:],
                        bass.AP(tensor=scr.tensor, offset=scr.offset,
                                ap=[[0, HD]] + scr.ap[1:]))
                    nc.vector.tensor_mul(fsl, pf[0:HD, :], rbc[:])

            def phase_c(h, qc):
                    hp, poff = h // 2, (h % 2) * HD
                    acts = [kb for kb in range(NKB) if nat_cls[qc, kb] != SKIP]
                    if not acts:
                        return  # output buffers are pre-zeroed
                    ncols = ext_n[qc]
                    nend = max(acts) + 1
                    assert nend * KBLK >= ncols > (nend - 1) * KBLK
                    ar = arow.tile([128, S], F32, name="ar")
                    dp = sml.tile([128, NKB], F32, name="dp")
                    # pair consecutive blocks into 2-bank psum tiles
                    kb = 0
                    npart = 0
                    while kb < nend:
                        take = 2 if kb + 1 < nend else 1
                        pn = ps.tile([128, 1024], F32, name="pn", tag="nat")
                        width = 0
                        for t in range(take):
                            blk = kb + t
                            if nat_cls[qc, blk] == SKIP:
                                w = min((blk + 1) * KBLK, ncols) - blk * KBLK
                                if w > 0:
                                    nc.vector.memset(
                                        ar[:, blk * KBLK:blk * KBLK + w], 0.0)
                                continue
                            psl = pn[:, t * KBLK:(t + 1) * KBLK]
                            mixed = nat_cls[qc, blk] == MIXED
                            nc.tensor.matmul(
                                psl,
                                qT[poff:poff + HD, hp,
                                   qc * QC:(qc + 1) * QC],
                                kT[poff:poff + HD, hp,
                                   blk * KBLK:(blk + 1) * KBLK],
                                start=True, stop=not mixed)
                            if mixed:
                                bi = bias_idx_n[(qc, blk)]
                                if bias_resident:
                                    nc.tensor.matmul(psl, ident[:],
                                                     bnres[:, bi, :],
                                                     start=False, stop=True)
                                else:
                                    bb = btl.tile([128, KBLK], F32R, name="btn")
                                    nc.gpsimd.dma_start(bb[:], bn_d.ap()[bi])
                                    nc.tensor.matmul(psl, ident[:], bb[:],
                                                     start=False, stop=True)
                            width += 1
                        if width == 2:
                            w = min((kb + 2) * KBLK, ncols) - kb * KBLK
                            nc.scalar.activation(
                                ar[:, kb * KBLK:kb * KBLK + w], pn[:, 0:w],
                                AF.Exp, scale=SCALE,
                                accum_out=dp[:, npart:npart + 1])
                            npart += 1
                        elif width == 1:
                            blk = kb if nat_cls[qc, kb] != SKIP else kb + 1
                            w = min((blk + 1) * KBLK, ncols) - blk * KBLK
                            nc.scalar.activation(
                                ar[:, blk * KBLK:blk * KBLK + w],
                                pn[:, (blk - kb) * KBLK:(blk - kb) * KBLK + w],
                                AF.Exp, scale=SCALE,
                                accum_out=dp[:, npart:npart + 1])
                            npart += 1
                        kb += take
                    dsum = sml.tile([128, 1], F32, name="dsum")
                    if npart > 1:
                        nc.vector.reduce_sum(dsum[:], dp[:, 0:npart], axis=AX.X)
                    else:
                        nc.vector.tensor_copy(dsum[:], dp[:, 0:1])
                    nc.vector.reciprocal(dsum[:], dsum[:])
                    nc.vector.tensor_scalar_mul(
                        ar[:, 0:ncols], ar[:, 0:ncols], dsum[:])
                    if ncols > 1024:
                        half = (ncols // 2 + 127) & ~127
                        nc.sync.dma_start(
                            attn_d.ap()[h, qc * QC:(qc + 1) * QC, 0:half],
                            ar[:, 0:half])
                        nc.sync.dma_start(
                            attn_d.ap()[h, qc * QC:(qc + 1) * QC, half:ncols],
                            ar[:, half:ncols])
                    else:
                        nc.sync.dma_start(
                            attn_d.ap()[h, qc * QC:(qc + 1) * QC, 0:ncols],
                            ar[:, 0:ncols])

            def phase_d(qc):
                ot = osb.tile([128, D], F32, name="ot")
                for nh in range(2):
                    po = ps.tile([128, 512], F32, name="po", tag="mm")
                    for d2 in range(2):
                        nc.tensor.matmul(
                            po[:],
                            fT[:, d2, qc * QC:(qc + 1) * QC],
                            wo[:, d2, nh * 512:(nh + 1) * 512],
                            start=(d2 == 0), stop=(d2 == 1))
                    nc.vector.tensor_copy(ot[:, nh * 512:(nh + 1) * 512], po[:])
                nc.sync.dma_start(
                    outp_d.ap()[qc * QC:(qc + 1) * QC, :], ot[:])

            import os as _os
            _order = _os.environ.get("KORDER", "seq")
            if _order == "seq":
                for h in range(HG):
                    for j in range(NJB):
                        phase_b(h, j)
                for h in range(HG):
                    for qc in range(NQC):
                        phase_c(h, qc)
                for qc in range(NQC):
                    phase_d(qc)
            elif _order == "jb":
                for j in range(NJB):
                    for h in range(HG):
                        phase_b(h, j)
                for h in range(HG):
                    for qc in range(NQC):
                        phase_c(h, qc)
                for qc in range(NQC):
                    phase_d(qc)
            elif _order == "jbd":
                for j in range(NJB):
                    for h in range(HG):
                        phase_b(h, j)
                    for qc in range(4 * j, 4 * j + 4):
                        phase_d(qc)
                for h in range(HG):
                    for qc in range(NQC):
                        phase_c(h, qc)
            elif _order == "hseq":
                for h in range(HG):
                    for j in range(NJB):
                        phase_b(h, j)
                    for qc in range(NQC):
                        phase_c(h, qc)
                for qc in range(NQC):
                    phase_d(qc)
            else:
                for j in range(NJB):
                    for h in range(HG):
                        phase_b(h, j)
                    for h in range(HG):
                        for qc in range(4 * j, 4 * j + 4):
                            phase_c(h, qc)
                    for qc in range(4 * j, 4 * j + 4):
                        phase_d(qc)

    nc.compile()
    return nc


def kernel(qry, key, val, attn_mask, Wq, Wk, Wv, Wo):
    global LAST_EXEC_NS, LAST_RESULTS, bias_idx_n, bias_idx_t

    qry = np.asarray(qry, np.float32)
    key = np.asarray(key, np.float32)
    val = np.asarray(val, np.float32)
    mask = np.asarray(attn_mask).astype(bool)
    Wq, Wk, Wv, Wo = (np.asarray(w, np.float32) for w in (Wq, Wk, Wv, Wo))

    nat_cls, tct_cls = _classify(mask)

    # bias tiles for mixed tiles (shared across batches via the union
    # classification; per-core data still uses the core's own batch mask).
    # Deduped by content (consistent across both batches): a causal mask
    # has only 4 distinct diagonal-crossing patterns per layout.
    biasf = np.where(mask, np.float32(0), np.float32(NEG))  # [B, S, S]
    biasfT = np.ascontiguousarray(biasf.swapaxes(1, 2))

    bias_idx_n, bias_idx_t = {}, {}
    rep_n, rep_t = [], []  # representative (qc,kb)/(kt,j) per unique index
    uniq_n, uniq_t = {}, {}
    for qc in range(NQC):
        for kb in range(NKB):
            if nat_cls[qc, kb] == MIXED:
                t0 = biasf[0, qc * QC:(qc + 1) * QC, kb * KBLK:(kb + 1) * KBLK]
                t1 = biasf[1, qc * QC:(qc + 1) * QC, kb * KBLK:(kb + 1) * KBLK]
                k = (t0.tobytes(), t1.tobytes())
                if k not in uniq_n:
                    uniq_n[k] = len(uniq_n)
                    rep_n.append((qc, kb))
                bias_idx_n[(qc, kb)] = uniq_n[k]
    for kt in range(NKT):
        for j in range(NJB):
            if tct_cls[kt, j] == MIXED:
                t0 = biasfT[0, kt * KT:(kt + 1) * KT, j * JB:(j + 1) * JB]
                t1 = biasfT[1, kt * KT:(kt + 1) * KT, j * JB:(j + 1) * JB]
                k = (t0.tobytes(), t1.tobytes())
                if k not in uniq_t:
                    uniq_t[k] = len(uniq_t)
                    rep_t.append((kt, j))
                bias_idx_t[(kt, j)] = uniq_t[k]
    n_bn, n_bt = len(rep_n), len(rep_t)
    bias_resident = (n_bn + n_bt) <= 16

    # exact active column extent per natural sq-chunk (union over batches)
    anycol = mask.any(axis=0)  # [S, S]
    ext_n = []
    for qc in range(NQC):
        rows = anycol[qc * QC:(qc + 1) * QC]
        nz = np.flatnonzero(rows.any(axis=0))
        ext_n.append(int(nz[-1]) + 1 if nz.size else 0)
    # active sq-column range per transposed mixed tile
    anyT = anycol.T
    rng_t = {}
    for (kt, j) in bias_idx_t:
        tl = anyT[kt * KT:(kt + 1) * KT, j * JB:(j + 1) * JB]
        nz = np.flatnonzero(tl.any(axis=0))
        if nz.size:
            rng_t[(kt, j)] = (int(nz[0]), int(nz[-1]) + 1)
        else:
            rng_t[(kt, j)] = (0, JB)

    cache_key = (nat_cls.tobytes(), tct_cls.tobytes(), bias_resident,
                 tuple(ext_n), tuple(sorted(rng_t.items())))
    if cache_key not in _prog_cache:
        _prog_cache[cache_key] = _build(nat_cls, tct_cls, n_bn, n_bt,
                                        bias_resident, ext_n, rng_t)
    nc = _prog_cache[cache_key]

    # per-batch host prep
    in_maps = []
    for c in range(NCORES):
        b, g = c // 4, c % 4
        cs = slice(DC * g, DC * g + DC)
        m = {
            "ident": np.eye(128, dtype=np.float32),
            "qryT": np.ascontiguousarray(qry[b].T),
            "keyT": np.ascontiguousarray(key[b].T),
            "valT": np.ascontiguousarray(val[b].T),
            "wqT": np.ascontiguousarray(Wq.T[:, cs]),
            "wkT": np.ascontiguousarray(Wk.T[:, cs]),
            "wvT": np.ascontiguousarray(Wv.T[:, cs]),
            "woT": np.ascontiguousarray(Wo.T[cs, :]),
        }
        if n_bn:
            bn = np.empty((n_bn, QC, KBLK), np.float32)
            for i, (qc, kb) in enumerate(rep_n):
                bn[i] = biasf[b, qc * QC:(qc + 1) * QC,
                              kb * KBLK:(kb + 1) * KBLK]
            m["biasN"] = bn
        if n_bt:
            bt = np.empty((n_bt, KT, JB), np.float32)
            for i, (kt, j) in enumerate(rep_t):
                bt[i] = biasfT[b, kt * KT:(kt + 1) * KT, j * JB:(j + 1) * JB]
            m["biasT"] = bt
        in_maps.append(m)

    res = run_bass_kernel_spmd(nc, in_maps, list(range(NCORES)))
    LAST_EXEC_NS = res.exec_time_ns
    LAST_RESULTS = res
    globals()["last_nc"] = nc
    globals()["last_in_maps"] = in_maps

    attn_weights = np.empty((B, H, S, S), np.float32)
    output64 = np.zeros((B, S, D), np.float64)
    for c in range(NCORES):
        b, g = c // 4, c % 4
        attn_weights[b, HG * g:HG * g + HG] = res.results[c]["attn"]
        output64[b] += res.results[c]["outp"]
    output = output64.astype(np.float32)
    return output, attn_weights


# revision 40
# speedup vs baseline: 319.8790x; 1.0038x over previous
"""Multi-head attention (B=2, S=2048, D=1024, H=16) on 8 Trainium2 cores.

Sharding: core c handles batch b=c//4 and head group g=c%4 (4 heads,
d_model slice of 256). Q/K/V/O projections are tensor-parallel over the
head dimension; attention is fully local per core; the output projection
produces per-core partial sums over d_model that the host reduces
(row-parallel W_o, "all-reduce" done in the unshard step).

Returns (output [B,S,D] fp32, attn_weights [B,H,S,S] fp32), matching the
reference module.
"""

import os
import numpy as np

import concourse.bass as bass
import concourse.bacc as bacc
import concourse.mybir as mybir
import concourse.tile as tile
from concourse.bass_utils import run_bass_kernel_spmd

F32 = mybir.dt.float32
F32R = mybir.dt.float32r
AF = mybir.ActivationFunctionType
AX = mybir.AxisListType

B, S, D, H = 2, 2048, 1024, 16
HG = 4              # heads per core
DC = 256            # d_model slice per core (HG * 64)
HD = 64             # head dim
NCORES = 8
SCALE = 0.125       # 1/sqrt(head_dim)
NEG = -1.0e30

QC = 128            # natural-layout sq chunk (psum partitions)
KBLK = 512          # natural-layout sk block (psum free dim)
KT = 128            # transposed-layout sk tile (psum partitions)
JB = 512            # transposed-layout sq block (psum free dim)

NQC, NKB = S // QC, S // KBLK     # 16, 4
NKT, NJB = S // KT, S // JB       # 16, 4

SKIP, FULL, MIXED = 0, 1, 2

LAST_EXEC_NS = None
LAST_RESULTS = None

_prog_cache = {}


def _classify(mask):
    """Per-tile classification over BOTH batches (union-active /
    intersection-full), for the natural [QC x KBLK] and transposed
    [KT x JB] grids. mask: [B, S, S] bool."""
    m = mask.reshape(B, NQC, QC, NKB, KBLK)
    any_n = m.any(axis=(0, 2, 4))          # [NQC, NKB]
    all_n = m.all(axis=(0, 2, 4))
    nat = np.where(all_n, FULL, np.where(any_n, MIXED, SKIP)).astype(np.int32)

    mt = mask.swapaxes(1, 2).reshape(B, NKT, KT, NJB, JB)
    any_t = mt.any(axis=(0, 2, 4))         # [NKT, NJB]
    all_t = mt.all(axis=(0, 2, 4))
    tct = np.where(all_t, FULL, np.where(any_t, MIXED, SKIP)).astype(np.int32)
    return nat, tct


def _build(nat_cls, tct_cls, n_bias_n, n_bias_t, bias_resident, ext_n, rng_t):
    """Build + compile the per-core program. All cores run the same
    program; per-core data differences come through the input tensors."""
    nc = bacc.Bacc("TRN2", target_bir_lowering=False, debug=False,
                   num_devices=NCORES)

    qryT_d = nc.dram_tensor("qryT", [D, S], F32R, kind="ExternalInput")
    keyT_d = nc.dram_tensor("keyT", [D, S], F32R, kind="ExternalInput")
    valT_d = nc.dram_tensor("valT", [D, S], F32R, kind="ExternalInput")
    wqT_d = nc.dram_tensor("wqT", [D, DC], F32R, kind="ExternalInput")
    wkT_d = nc.dram_tensor("wkT", [D, DC], F32R, kind="ExternalInput")
    wvT_d = nc.dram_tensor("wvT", [D, DC], F32R, kind="ExternalInput")
    woT_d = nc.dram_tensor("woT", [DC, D], F32R, kind="ExternalInput")
    bn_d = bt_d = None
    if n_bias_n:
        bn_d = nc.dram_tensor("biasN", [n_bias_n, QC, KBLK], F32R,
                              kind="ExternalInput")
    if n_bias_t:
        bt_d = nc.dram_tensor("biasT", [n_bias_t, KT, JB], F32R,
                              kind="ExternalInput")
    ident_d = nc.dram_tensor("ident", [128, 128], F32R, kind="ExternalInput")

    attn_d = nc.dram_tensor("attn", [HG, S, S], F32, kind="ExternalOutput")
    outp_d = nc.dram_tensor("outp", [S, D], F32, kind="ExternalOutput")
    denr_d = nc.dram_tensor("denr", [HG * NJB, JB], F32)  # internal scratch

    r = F32R

    with tile.TileContext(nc) as tc:
        with (
            tc.tile_pool(name="res", bufs=1) as res,
            tc.tile_pool(name="inq", bufs=3) as inq,
            tc.tile_pool(name="inv", bufs=9) as inv,
            tc.tile_pool(name="ext", bufs=6) as extp,
            tc.tile_pool(name="arow", bufs=5) as arow,
            tc.tile_pool(name="btl", bufs=2) as btl,
            tc.tile_pool(name="sml", bufs=2) as sml,
            tc.tile_pool(name="osb", bufs=2) as osb,
            tc.tile_pool(name="ps", bufs=2, space="PSUM") as ps,
        ):
            # ---- resident SBUF tensors ----
            qT = res.tile([128, 2, S], F32R)     # q^T: d-slice rows, s cols
            kT = res.tile([128, 2, S], F32R)
            fT = res.tile([128, 2, S], F32R)     # normalized feats^T
            v1s = res.tile([128, NKT, HG, HD + 1], F32R)  # [v | ones] per sk-chunk
            wq = res.tile([128, 8, DC], F32R)
            wk = res.tile([128, 8, DC], F32R)
            wv = res.tile([128, 8, DC], F32R)
            wo = res.tile([128, 2, D], F32R)
            ident = res.tile([128, 128], F32R)
            nc.sync.dma_start(ident[:], ident_d.ap())
            if bias_resident:
                bnres = res.tile([128, max(n_bias_n, 1), KBLK], F32R)
                btres = res.tile([128, max(n_bias_t, 1), JB], F32R)
            # fp32r tiles cannot be memset directly; write the denominator
            # ones-column via a rounding copy from an fp32 ones tile
            ones_c = res.tile([128, NKT * HG], F32)
            nc.vector.memset(ones_c[:], 1.0)
            nc.vector.tensor_copy(
                v1s[:, :, :, HD],
                ones_c[:].rearrange("p (a b) -> p a b", b=HG))

            nc.sync.dma_start(wq[:], wqT_d.ap().rearrange("(c p) n -> p c n", p=128))
            nc.sync.dma_start(wk[:], wkT_d.ap().rearrange("(c p) n -> p c n", p=128))
            nc.sync.dma_start(wv[:], wvT_d.ap().rearrange("(c p) n -> p c n", p=128))
            nc.sync.dma_start(wo[:], woT_d.ap().rearrange("(c p) n -> p c n", p=128))
            if bias_resident:
                if n_bias_n:
                    nc.sync.dma_start(
                        bnres[:, 0:n_bias_n, :],
                        bn_d.ap().rearrange("t p n -> p t n"))
                if n_bias_t:
                    nc.sync.dma_start(
                        btres[:, 0:n_bias_t, :],
                        bt_d.ap().rearrange("t p n -> p t n"))

            # ---- Phase A: projections, interleaved by s-block so phase B
            # can start after the first block ----
            for sb in range(S // 512):
                # q^T / k^T: [d_c, S] = (W.T slice).T @ x^T, over D chunks
                for (w_s, x_d, dst, tag) in ((wq, qryT_d, qT, "mm"),
                                             (wk, keyT_d, kT, "nat")):
                    psums = []
                    for hp in range(2):
                        p = ps.tile([128, 512], F32, name=f"pp{hp}", tag=tag)
                        psums.append(p)
                    for dc in range(8):
                        xin = inq.tile([128, 512], F32R, name="xin")
                        nc.sync.dma_start(
                            xin[:],
                            x_d.ap()[dc * 128:(dc + 1) * 128,
                                     sb * 512:(sb + 1) * 512])
                        for hp in range(2):
                            nc.tensor.matmul(
                                psums[hp][:],
                                w_s[:, dc, hp * 128:(hp + 1) * 128],
                                xin[:],
                                start=(dc == 0), stop=(dc == 7))
                    for hp in range(2):
                        nc.vector.tensor_copy(
                            dst[:, hp, sb * 512:(sb + 1) * 512], psums[hp][:])

                # v natural layout: [s, d_c] = (val^T tile).T @ (W_v.T slice)
                vins = []
                for dc in range(8):
                    vin = inv.tile([128, 512], F32R, name="vin")
                    nc.sync.dma_start(
                        vin[:],
                        valT_d.ap()[dc * 128:(dc + 1) * 128,
                                    sb * 512:(sb + 1) * 512])
                    vins.append(vin)
                for s4 in range(4):
                    sc = sb * 4 + s4
                    pv = ps.tile([128, DC], F32, name="pv", tag="ft")
                    for dc in range(8):
                        nc.tensor.matmul(
                            pv[:], vins[dc][:, s4 * 128:(s4 + 1) * 128],
                            wv[:, dc, :],
                            start=(dc == 0), stop=(dc == 7))
                    nc.vector.tensor_copy(
                        v1s[:, sc, :, 0:HD],
                        pv[:].rearrange("p (h d) -> p h d", h=HG))

            # ---- Phases B/C/D interleaved per sq-block j ----
            # B: transposed scores -> exp -> feats^T (+denom)
            # C: natural scores -> exp(+accum) -> normalize -> attn DMA
            # D: partial output projection (needs all heads' feats for j)
            def phase_b(h, j):
                    hp, poff = h // 2, (h % 2) * HD
                    acts = [kt for kt in range(NKT) if tct_cls[kt, j] != SKIP]
                    fsl = fT[poff:poff + HD, hp, j * JB:(j + 1) * JB]
                    if not acts:
                        nc.vector.memset(fsl, 0.0)
                        return
                    pf = ps.tile([HD + 1, JB], F32, name="pf", tag="ft")
                    qsl = qT[poff:poff + HD, hp, j * JB:(j + 1) * JB]
                    nacts = len(acts)
                    for i, kt in enumerate(acts):
                        mixed = tct_cls[kt, j] == MIXED
                        c0, c1 = 0, JB
                        if mixed and i > 0:
                            c0, c1 = rng_t[(kt, j)]
                        w = c1 - c0
                        pst = ps.tile([128, JB], F32, name="pst", tag="mm")
                        nc.tensor.matmul(
                            pst[:, c0:c1],
                            kT[poff:poff + HD, hp,
                               kt * KT:(kt + 1) * KT],
                            qsl[:, c0:c1], start=True, stop=not mixed)
                        if mixed:
                            bi = bias_idx_t[(kt, j)]
                            if bias_resident:
                                nc.tensor.matmul(pst[:, c0:c1], ident[:],
                                                 btres[:, bi, c0:c1],
                                                 start=False, stop=True)
                            else:
                                bb = btl.tile([128, JB], F32R, name="btt")
                                nc.gpsimd.dma_start(bb[:], bt_d.ap()[bi])
                                nc.tensor.matmul(pst[:, c0:c1], ident[:],
                                                 bb[:, c0:c1],
                                                 start=False, stop=True)
                        ext = extp.tile([128, JB], F32R, name="ext")
                        nc.scalar.activation(ext[:, c0:c1], pst[:, c0:c1],
                                             AF.Exp, scale=SCALE)
                        nc.tensor.matmul(
                            pf[:, c0:c1], v1s[:, kt, h, :], ext[:, c0:c1],
                            start=(i == 0), stop=(i == nacts - 1))
                    # denominator row -> reciprocal -> PE ones-broadcast
                    rr = sml.tile([1, JB], F32, name="rr")
                    nc.vector.reciprocal(rr[:], pf[HD:HD + 1, :])
                    scr = denr_d.ap()[h * NJB + j:h * NJB + j + 1, :]
                    nc.gpsimd.dma_start(scr, rr[:])
                    rbc = sml.tile([HD, JB], F32, name="rbc")
                    nc.gpsimd.dma_start(
                        rbc[:],
                        bass.AP(tensor=scr.tensor, offset=scr.offset,
                                ap=[[0, HD]] + scr.ap[1:]))
                    nc.vector.tensor_mul(fsl, pf[0:HD, :], rbc[:])

            def phase_c(h, qc):
                    hp, poff = h // 2, (h % 2) * HD
                    acts = [kb for kb in range(NKB) if nat_cls[qc, kb] != SKIP]
                    if not acts:
                        return  # output buffers are pre-zeroed
                    ncols = ext_n[qc]
                    nend = max(acts) + 1
                    assert nend * KBLK >= ncols > (nend - 1) * KBLK
                    ar = arow.tile([128, S], F32, name="ar")
                    dp = sml.tile([128, NKB], F32, name="dp")
                    # pair consecutive blocks into 2-bank psum tiles
                    kb = 0
                    npart = 0
                    while kb < nend:
                        take = 2 if kb + 1 < nend else 1
                        pn = ps.tile([128, 1024], F32, name="pn", tag="nat")
                        width = 0
                        for t in range(take):
                            blk = kb + t
                            if nat_cls[qc, blk] == SKIP:
                                w = min((blk + 1) * KBLK, ncols) - blk * KBLK
                                if w > 0:
                                    nc.vector.memset(
                                        ar[:, blk * KBLK:blk * KBLK + w], 0.0)
                                continue
                            psl = pn[:, t * KBLK:(t + 1) * KBLK]
                            mixed = nat_cls[qc, blk] == MIXED
                            nc.tensor.matmul(
                                psl,
                                qT[poff:poff + HD, hp,
                                   qc * QC:(qc + 1) * QC],
                                kT[poff:poff + HD, hp,
                                   blk * KBLK:(blk + 1) * KBLK],
                                start=True, stop=not mixed)
                            if mixed:
                                bi = bias_idx_n[(qc, blk)]
                                if bias_resident:
                                    nc.tensor.matmul(psl, ident[:],
                                                     bnres[:, bi, :],
                                                     start=False, stop=True)
                                else:
                                    bb = btl.tile([128, KBLK], F32R, name="btn")
                                    nc.gpsimd.dma_start(bb[:], bn_d.ap()[bi])
                                    nc.tensor.matmul(psl, ident[:], bb[:],
                                                     start=False, stop=True)
                            width += 1
                        if width == 2:
                            w = min((kb + 2) * KBLK, ncols) - kb * KBLK
                            nc.scalar.activation(
                                ar[:, kb * KBLK:kb * KBLK + w], pn[:, 0:w],
                                AF.Exp, scale=SCALE,
                                accum_out=dp[:, npart:npart + 1])
                            npart += 1
                        elif width == 1:
                            blk = kb if nat_cls[qc, kb] != SKIP else kb + 1
                            w = min((blk + 1) * KBLK, ncols) - blk * KBLK
                            nc.scalar.activation(
                                ar[:, blk * KBLK:blk * KBLK + w],
                                pn[:, (blk - kb) * KBLK:(blk - kb) * KBLK + w],
                                AF.Exp, scale=SCALE,
                                accum_out=dp[:, npart:npart + 1])
                            npart += 1
                        kb += take
                    dsum = sml.tile([128, 1], F32, name="dsum")
                    if npart > 1:
                        nc.vector.reduce_sum(dsum[:], dp[:, 0:npart], axis=AX.X)
                    else:
                        nc.vector.tensor_copy(dsum[:], dp[:, 0:1])
                    nc.vector.reciprocal(dsum[:], dsum[:])
                    nc.vector.tensor_scalar_mul(
                        ar[:, 0:ncols], ar[:, 0:ncols], dsum[:])
                    nparts = max(1, min(4, ncols // 512))
                    step = ((ncols + nparts - 1) // nparts + 127) & ~127
                    c = 0
                    while c < ncols:
                        ce = min(c + step, ncols)
                        nc.sync.dma_start(
                            attn_d.ap()[h, qc * QC:(qc + 1) * QC, c:ce],
                            ar[:, c:ce])
                        c = ce

            def phase_d(qc):
                ot = osb.tile([128, D], F32, name="ot")
                for nh in range(2):
                    po = ps.tile([128, 512], F32, name="po", tag="mm")
                    for d2 in range(2):
                        nc.tensor.matmul(
                            po[:],
                            fT[:, d2, qc * QC:(qc + 1) * QC],
                            wo[:, d2, nh * 512:(nh + 1) * 512],
                            start=(d2 == 0), stop=(d2 == 1))
                    nc.vector.tensor_copy(ot[:, nh * 512:(nh + 1) * 512], po[:])
                nc.sync.dma_start(
                    outp_d.ap()[qc * QC:(qc + 1) * QC, :], ot[:])

            import os as _os
            _order = _os.environ.get("KORDER", "seq")
            if _order == "seq":
                for h in range(HG):
                    for j in range(NJB):
                        phase_b(h, j)
                for h in range(HG):
                    for qc in range(NQC):
                        phase_c(h, qc)
                for qc in range(NQC):
                    phase_d(qc)
            elif _order == "jb":
                for j in range(NJB):
                    for h in range(HG):
                        phase_b(h, j)
                for h in range(HG):
                    for qc in range(NQC):
                        phase_c(h, qc)
                for qc in range(NQC):
                    phase_d(qc)
            elif _order == "jbd":
                for j in range(NJB):
                    for h in range(HG):
                        phase_b(h, j)
                    for qc in range(4 * j, 4 * j + 4):
                        phase_d(qc)
                for h in range(HG):
                    for qc in range(NQC):
                        phase_c(h, qc)
            elif _order == "hseq":
                for h in range(HG):
                    for j in range(NJB):
                        phase_b(h, j)
                    for qc in range(NQC):
                        phase_c(h, qc)
                for qc in range(NQC):
                    phase_d(qc)
            else:
                for j in range(NJB):
                    for h in range(HG):
                        phase_b(h, j)
                    for h in range(HG):
                        for qc in range(4 * j, 4 * j + 4):
                            phase_c(h, qc)
                    for qc in range(4 * j, 4 * j + 4):
                        phase_d(qc)

    nc.compile()
    return nc


def kernel(qry, key, val, attn_mask, Wq, Wk, Wv, Wo):
    global LAST_EXEC_NS, LAST_RESULTS, bias_idx_n, bias_idx_t

    qry = np.asarray(qry, np.float32)
    key = np.asarray(key, np.float32)
    val = np.asarray(val, np.float32)
    mask = np.asarray(attn_mask).astype(bool)
    Wq, Wk, Wv, Wo = (np.asarray(w, np.float32) for w in (Wq, Wk, Wv, Wo))

    nat_cls, tct_cls = _classify(mask)

    # bias tiles for mixed tiles (shared across batches via the union
    # classification; per-core data still uses the core's own batch mask).
    # Deduped by content (consistent across both batches): a causal mask
    # has only 4 distinct diagonal-crossing patterns per layout.
    biasf = np.where(mask, np.float32(0), np.float32(NEG))  # [B, S, S]
    biasfT = np.ascontiguousarray(biasf.swapaxes(1, 2))

    bias_idx_n, bias_idx_t = {}, {}
    rep_n, rep_t = [], []  # representative (qc,kb)/(kt,j) per unique index
    uniq_n, uniq_t = {}, {}
    for qc in range(NQC):
        for kb in range(NKB):
            if nat_cls[qc, kb] == MIXED:
                t0 = biasf[0, qc * QC:(qc + 1) * QC, kb * KBLK:(kb + 1) * KBLK]
                t1 = biasf[1, qc * QC:(qc + 1) * QC, kb * KBLK:(kb + 1) * KBLK]
                k = (t0.tobytes(), t1.tobytes())
                if k not in uniq_n:
                    uniq_n[k] = len(uniq_n)
                    rep_n.append((qc, kb))
                bias_idx_n[(qc, kb)] = uniq_n[k]
    for kt in range(NKT):
        for j in range(NJB):
            if tct_cls[kt, j] == MIXED:
                t0 = biasfT[0, kt * KT:(kt + 1) * KT, j * JB:(j + 1) * JB]
                t1 = biasfT[1, kt * KT:(kt + 1) * KT, j * JB:(j + 1) * JB]
                k = (t0.tobytes(), t1.tobytes())
                if k not in uniq_t:
                    uniq_t[k] = len(uniq_t)
                    rep_t.append((kt, j))
                bias_idx_t[(kt, j)] = uniq_t[k]
    n_bn, n_bt = len(rep_n), len(rep_t)
    bias_resident = (n_bn + n_bt) <= 16

    # exact active column extent per natural sq-chunk (union over batches)
    anycol = mask.any(axis=0)  # [S, S]
    ext_n = []
    for qc in range(NQC):
        rows = anycol[qc * QC:(qc + 1) * QC]
        nz = np.flatnonzero(rows.any(axis=0))
        ext_n.append(int(nz[-1]) + 1 if nz.size else 0)
    # active sq-column range per transposed mixed tile
    anyT = anycol.T
    rng_t = {}
    for (kt, j) in bias_idx_t:
        tl = anyT[kt * KT:(kt + 1) * KT, j * JB:(j + 1) * JB]
        nz = np.flatnonzero(tl.any(axis=0))
        if nz.size:
            rng_t[(kt, j)] = (int(nz[0]), int(nz[-1]) + 1)
        else:
            rng_t[(kt, j)] = (0, JB)

    cache_key = (nat_cls.tobytes(), tct_cls.tobytes(), bias_resident,
                 tuple(ext_n), tuple(sorted(rng_t.items())))
    if cache_key not in _prog_cache:
        _prog_cache[cache_key] = _build(nat_cls, tct_cls, n_bn, n_bt,
                                        bias_resident, ext_n, rng_t)
    nc = _prog_cache[cache_key]

    # per-batch host prep
    in_maps = []
    for c in range(NCORES):
        b, g = c // 4, c % 4
        cs = slice(DC * g, DC * g + DC)
        m = {
            "ident": np.eye(128, dtype=np.float32),
            "qryT": np.ascontiguousarray(qry[b].T),
            "keyT": np.ascontiguousarray(key[b].T),
            "valT": np.ascontiguousarray(val[b].T),
            "wqT": np.ascontiguousarray(Wq.T[:, cs]),
            "wkT": np.ascontiguousarray(Wk.T[:, cs]),
            "wvT": np.ascontiguousarray(Wv.T[:, cs]),
            "woT": np.ascontiguousarray(Wo.T[cs, :]),
        }
        if n_bn:
            bn = np.empty((n_bn, QC, KBLK), np.float32)
            for i, (qc, kb) in enumerate(rep_n):
                bn[i] = biasf[b, qc * QC:(qc + 1) * QC,
                              kb * KBLK:(kb + 1) * KBLK]
            m["biasN"] = bn
        if n_bt:
            bt = np.empty((n_bt, KT, JB), np.float32)
            for i, (kt, j) in enumerate(rep_t):
                bt[i] = biasfT[b, kt * KT:(kt + 1) * KT, j * JB:(j + 1) * JB]
            m["biasT"] = bt
        in_maps.append(m)

    res = run_bass_kernel_spmd(nc, in_maps, list(range(NCORES)))
    LAST_EXEC_NS = res.exec_time_ns
    LAST_RESULTS = res
    globals()["last_nc"] = nc
    globals()["last_in_maps"] = in_maps

    attn_weights = np.empty((B, H, S, S), np.float32)
    output64 = np.zeros((B, S, D), np.float64)
    for c in range(NCORES):
        b, g = c // 4, c % 4
        attn_weights[b, HG * g:HG * g + HG] = res.results[c]["attn"]
        output64[b] += res.results[c]["outp"]
    output = output64.astype(np.float32)
    return output, attn_weights


# revision 41
# speedup vs baseline: 321.4209x; 1.0048x over previous
"""Multi-head attention (B=2, S=2048, D=1024, H=16) on 8 Trainium2 cores.

Sharding: core c handles batch b=c//4 and head group g=c%4 (4 heads,
d_model slice of 256). Q/K/V/O projections are tensor-parallel over the
head dimension; attention is fully local per core; the output projection
produces per-core partial sums over d_model that the host reduces
(row-parallel W_o, "all-reduce" done in the unshard step).

Returns (output [B,S,D] fp32, attn_weights [B,H,S,S] fp32), matching the
reference module.
"""

import os
import numpy as np

import concourse.bass as bass
import concourse.bacc as bacc
import concourse.mybir as mybir
import concourse.tile as tile
from concourse.bass_utils import run_bass_kernel_spmd

F32 = mybir.dt.float32
F32R = mybir.dt.float32r
AF = mybir.ActivationFunctionType
AX = mybir.AxisListType

B, S, D, H = 2, 2048, 1024, 16
HG = 4              # heads per core
DC = 256            # d_model slice per core (HG * 64)
HD = 64             # head dim
NCORES = 8
SCALE = 0.125       # 1/sqrt(head_dim)
NEG = -1.0e30

QC = 128            # natural-layout sq chunk (psum partitions)
KBLK = 512          # natural-layout sk block (psum free dim)
KT = 128            # transposed-layout sk tile (psum partitions)
JB = 512            # transposed-layout sq block (psum free dim)

NQC, NKB = S // QC, S // KBLK     # 16, 4
NKT, NJB = S // KT, S // JB       # 16, 4

SKIP, FULL, MIXED = 0, 1, 2

LAST_EXEC_NS = None
LAST_RESULTS = None

_prog_cache = {}


def _classify(mask):
    """Per-tile classification over BOTH batches (union-active /
    intersection-full), for the natural [QC x KBLK] and transposed
    [KT x JB] grids. mask: [B, S, S] bool."""
    m = mask.reshape(B, NQC, QC, NKB, KBLK)
    any_n = m.any(axis=(0, 2, 4))          # [NQC, NKB]
    all_n = m.all(axis=(0, 2, 4))
    nat = np.where(all_n, FULL, np.where(any_n, MIXED, SKIP)).astype(np.int32)

    mt = mask.swapaxes(1, 2).reshape(B, NKT, KT, NJB, JB)
    any_t = mt.any(axis=(0, 2, 4))         # [NKT, NJB]
    all_t = mt.all(axis=(0, 2, 4))
    tct = np.where(all_t, FULL, np.where(any_t, MIXED, SKIP)).astype(np.int32)
    return nat, tct


def _build(nat_cls, tct_cls, n_bias_n, n_bias_t, bias_resident, ext_n, rng_t):
    """Build + compile the per-core program. All cores run the same
    program; per-core data differences come through the input tensors."""
    nc = bacc.Bacc("TRN2", target_bir_lowering=False, debug=False,
                   num_devices=NCORES)

    qryT_d = nc.dram_tensor("qryT", [D, S], F32R, kind="ExternalInput")
    keyT_d = nc.dram_tensor("keyT", [D, S], F32R, kind="ExternalInput")
    valT_d = nc.dram_tensor("valT", [D, S], F32R, kind="ExternalInput")
    wqT_d = nc.dram_tensor("wqT", [D, DC], F32R, kind="ExternalInput")
    wkT_d = nc.dram_tensor("wkT", [D, DC], F32R, kind="ExternalInput")
    wvT_d = nc.dram_tensor("wvT", [D, DC], F32R, kind="ExternalInput")
    woT_d = nc.dram_tensor("woT", [DC, D], F32R, kind="ExternalInput")
    bn_d = bt_d = None
    if n_bias_n:
        bn_d = nc.dram_tensor("biasN", [n_bias_n, QC, KBLK], F32R,
                              kind="ExternalInput")
    if n_bias_t:
        bt_d = nc.dram_tensor("biasT", [n_bias_t, KT, JB], F32R,
                              kind="ExternalInput")
    ident_d = nc.dram_tensor("ident", [128, 128], F32R, kind="ExternalInput")

    attn_d = nc.dram_tensor("attn", [HG, S, S], F32, kind="ExternalOutput")
    outp_d = nc.dram_tensor("outp", [S, D], F32, kind="ExternalOutput")
    denr_d = nc.dram_tensor("denr", [HG * NJB, JB], F32)  # internal scratch

    r = F32R

    with tile.TileContext(nc) as tc:
        with (
            tc.tile_pool(name="res", bufs=1) as res,
            tc.tile_pool(name="inq", bufs=3) as inq,
            tc.tile_pool(name="inv", bufs=9) as inv,
            tc.tile_pool(name="ext", bufs=6) as extp,
            tc.tile_pool(name="arow", bufs=5) as arow,
            tc.tile_pool(name="btl", bufs=2) as btl,
            tc.tile_pool(name="sml", bufs=2) as sml,
            tc.tile_pool(name="osb", bufs=2) as osb,
            tc.tile_pool(name="ps", bufs=2, space="PSUM") as ps,
        ):
            # ---- resident SBUF tensors ----
            qT = res.tile([128, 2, S], F32R)     # q^T: d-slice rows, s cols
            kT = res.tile([128, 2, S], F32R)
            fT = res.tile([128, 2, S], F32R)     # normalized feats^T
            v1s = res.tile([128, NKT, HG, HD + 1], F32R)  # [v | ones] per sk-chunk
            wq = res.tile([128, 8, DC], F32R)
            wk = res.tile([128, 8, DC], F32R)
            wv = res.tile([128, 8, DC], F32R)
            wo = res.tile([128, 2, D], F32R)
            ident = res.tile([128, 128], F32R)
            nc.sync.dma_start(ident[:], ident_d.ap())
            if bias_resident:
                bnres = res.tile([128, max(n_bias_n, 1), KBLK], F32R)
                btres = res.tile([128, max(n_bias_t, 1), JB], F32R)
            # fp32r tiles cannot be memset directly; write the denominator
            # ones-column via a rounding copy from an fp32 ones tile
            ones_c = res.tile([128, NKT * HG], F32)
            nc.vector.memset(ones_c[:], 1.0)
            nc.vector.tensor_copy(
                v1s[:, :, :, HD],
                ones_c[:].rearrange("p (a b) -> p a b", b=HG))

            nc.sync.dma_start(wq[:], wqT_d.ap().rearrange("(c p) n -> p c n", p=128))
            nc.sync.dma_start(wk[:], wkT_d.ap().rearrange("(c p) n -> p c n", p=128))
            nc.sync.dma_start(wv[:], wvT_d.ap().rearrange("(c p) n -> p c n", p=128))
            nc.sync.dma_start(wo[:], woT_d.ap().rearrange("(c p) n -> p c n", p=128))
            if bias_resident:
                if n_bias_n:
                    nc.sync.dma_start(
                        bnres[:, 0:n_bias_n, :],
                        bn_d.ap().rearrange("t p n -> p t n"))
                if n_bias_t:
                    nc.sync.dma_start(
                        btres[:, 0:n_bias_t, :],
                        bt_d.ap().rearrange("t p n -> p t n"))

            # ---- Phase A: projections, interleaved by s-block so phase B
            # can start after the first block ----
            for sb in range(S // 512):
                # q^T / k^T: [d_c, S] = (W.T slice).T @ x^T, over D chunks
                for (w_s, x_d, dst, tag) in ((wq, qryT_d, qT, "mm"),
                                             (wk, keyT_d, kT, "nat")):
                    psums = []
                    for hp in range(2):
                        p = ps.tile([128, 512], F32, name=f"pp{hp}", tag=tag)
                        psums.append(p)
                    for dc in range(8):
                        xin = inq.tile([128, 512], F32R, name="xin")
                        nc.sync.dma_start(
                            xin[:],
                            x_d.ap()[dc * 128:(dc + 1) * 128,
                                     sb * 512:(sb + 1) * 512])
                        for hp in range(2):
                            nc.tensor.matmul(
                                psums[hp][:],
                                w_s[:, dc, hp * 128:(hp + 1) * 128],
                                xin[:],
                                start=(dc == 0), stop=(dc == 7))
                    for hp in range(2):
                        nc.vector.tensor_copy(
                            dst[:, hp, sb * 512:(sb + 1) * 512], psums[hp][:])

                # v natural layout: [s, d_c] = (val^T tile).T @ (W_v.T slice)
                vins = []
                for dc in range(8):
                    vin = inv.tile([128, 512], F32R, name="vin")
                    nc.sync.dma_start(
                        vin[:],
                        valT_d.ap()[dc * 128:(dc + 1) * 128,
                                    sb * 512:(sb + 1) * 512])
                    vins.append(vin)
                for s4 in range(4):
                    sc = sb * 4 + s4
                    pv = ps.tile([128, DC], F32, name="pv", tag="ft")
                    for dc in range(8):
                        nc.tensor.matmul(
                            pv[:], vins[dc][:, s4 * 128:(s4 + 1) * 128],
                            wv[:, dc, :],
                            start=(dc == 0), stop=(dc == 7))
                    nc.vector.tensor_copy(
                        v1s[:, sc, :, 0:HD],
                        pv[:].rearrange("p (h d) -> p h d", h=HG))

            # ---- Phases B/C/D interleaved per sq-block j ----
            # B: transposed scores -> exp -> feats^T (+denom)
            # C: natural scores -> exp(+accum) -> normalize -> attn DMA
            # D: partial output projection (needs all heads' feats for j)
            def phase_b(h, j):
                    hp, poff = h // 2, (h % 2) * HD
                    acts = [kt for kt in range(NKT) if tct_cls[kt, j] != SKIP]
                    fsl = fT[poff:poff + HD, hp, j * JB:(j + 1) * JB]
                    if not acts:
                        nc.vector.memset(fsl, 0.0)
                        return
                    pf = ps.tile([HD + 1, JB], F32, name="pf", tag="ft")
                    qsl = qT[poff:poff + HD, hp, j * JB:(j + 1) * JB]
                    nacts = len(acts)
                    for i, kt in enumerate(acts):
                        mixed = tct_cls[kt, j] == MIXED
                        c0, c1 = 0, JB
                        if mixed and i > 0:
                            c0, c1 = rng_t[(kt, j)]
                        w = c1 - c0
                        pst = ps.tile([128, JB], F32, name="pst", tag="mm")
                        nc.tensor.matmul(
                            pst[:, c0:c1],
                            kT[poff:poff + HD, hp,
                               kt * KT:(kt + 1) * KT],
                            qsl[:, c0:c1], start=True, stop=not mixed)
                        if mixed:
                            bi = bias_idx_t[(kt, j)]
                            if bias_resident:
                                nc.tensor.matmul(pst[:, c0:c1], ident[:],
                                                 btres[:, bi, c0:c1],
                                                 start=False, stop=True)
                            else:
                                bb = btl.tile([128, JB], F32R, name="btt")
                                nc.gpsimd.dma_start(bb[:], bt_d.ap()[bi])
                                nc.tensor.matmul(pst[:, c0:c1], ident[:],
                                                 bb[:, c0:c1],
                                                 start=False, stop=True)
                        ext = extp.tile([128, JB], F32R, name="ext")
                        nc.scalar.activation(ext[:, c0:c1], pst[:, c0:c1],
                                             AF.Exp, scale=SCALE)
                        nc.tensor.matmul(
                            pf[:, c0:c1], v1s[:, kt, h, :], ext[:, c0:c1],
                            start=(i == 0), stop=(i == nacts - 1))
                    # denominator row -> reciprocal -> PE ones-broadcast
                    rr = sml.tile([1, JB], F32, name="rr")
                    nc.vector.reciprocal(rr[:], pf[HD:HD + 1, :])
                    scr = denr_d.ap()[h * NJB + j:h * NJB + j + 1, :]
                    nc.gpsimd.dma_start(scr, rr[:])
                    rbc = sml.tile([HD, JB], F32, name="rbc")
                    nc.gpsimd.dma_start(
                        rbc[:],
                        bass.AP(tensor=scr.tensor, offset=scr.offset,
                                ap=[[0, HD]] + scr.ap[1:]))
                    nc.vector.tensor_mul(fsl, pf[0:HD, :], rbc[:])

            def phase_c(h, qc):
                    hp, poff = h // 2, (h % 2) * HD
                    acts = [kb for kb in range(NKB) if nat_cls[qc, kb] != SKIP]
                    if not acts:
                        return  # output buffers are pre-zeroed
                    ncols = ext_n[qc]
                    nend = max(acts) + 1
                    assert nend * KBLK >= ncols > (nend - 1) * KBLK
                    ar = arow.tile([128, S], F32, name="ar")
                    dp = sml.tile([128, NKB], F32, name="dp")
                    # pair consecutive blocks into 2-bank psum tiles
                    kb = 0
                    npart = 0
                    while kb < nend:
                        take = 2 if kb + 1 < nend else 1
                        pn = ps.tile([128, 1024], F32, name="pn", tag="nat")
                        width = 0
                        for t in range(take):
                            blk = kb + t
                            if nat_cls[qc, blk] == SKIP:
                                w = min((blk + 1) * KBLK, ncols) - blk * KBLK
                                if w > 0:
                                    nc.vector.memset(
                                        ar[:, blk * KBLK:blk * KBLK + w], 0.0)
                                continue
                            psl = pn[:, t * KBLK:(t + 1) * KBLK]
                            mixed = nat_cls[qc, blk] == MIXED
                            nc.tensor.matmul(
                                psl,
                                qT[poff:poff + HD, hp,
                                   qc * QC:(qc + 1) * QC],
                                kT[poff:poff + HD, hp,
                                   blk * KBLK:(blk + 1) * KBLK],
                                start=True, stop=not mixed)
                            if mixed:
                                bi = bias_idx_n[(qc, blk)]
                                if bias_resident:
                                    nc.tensor.matmul(psl, ident[:],
                                                     bnres[:, bi, :],
                                                     start=False, stop=True)
                                else:
                                    bb = btl.tile([128, KBLK], F32R, name="btn")
                                    nc.gpsimd.dma_start(bb[:], bn_d.ap()[bi])
                                    nc.tensor.matmul(psl, ident[:], bb[:],
                                                     start=False, stop=True)
                            width += 1
                        if width == 2:
                            w = min((kb + 2) * KBLK, ncols) - kb * KBLK
                            nc.scalar.activation(
                                ar[:, kb * KBLK:kb * KBLK + w], pn[:, 0:w],
                                AF.Exp, scale=SCALE,
                                accum_out=dp[:, npart:npart + 1])
                            npart += 1
                        elif width == 1:
                            blk = kb if nat_cls[qc, kb] != SKIP else kb + 1
                            w = min((blk + 1) * KBLK, ncols) - blk * KBLK
                            nc.scalar.activation(
                                ar[:, blk * KBLK:blk * KBLK + w],
                                pn[:, (blk - kb) * KBLK:(blk - kb) * KBLK + w],
                                AF.Exp, scale=SCALE,
                                accum_out=dp[:, npart:npart + 1])
                            npart += 1
                        kb += take
                    dsum = sml.tile([128, 1], F32, name="dsum")
                    if npart > 1:
                        nc.vector.reduce_sum(dsum[:], dp[:, 0:npart], axis=AX.X)
                    else:
                        nc.vector.tensor_copy(dsum[:], dp[:, 0:1])
                    nc.vector.reciprocal(dsum[:], dsum[:])
                    nc.vector.tensor_scalar_mul(
                        ar[:, 0:ncols], ar[:, 0:ncols], dsum[:])
                    nparts = max(1, min(4, ncols // 512))
                    step = ((ncols + nparts - 1) // nparts + 127) & ~127
                    c = 0
                    while c < ncols:
                        ce = min(c + step, ncols)
                        nc.sync.dma_start(
                            attn_d.ap()[h, qc * QC:(qc + 1) * QC, c:ce],
                            ar[:, c:ce])
                        c = ce

            def phase_d(qc):
                ot = osb.tile([128, D], F32, name="ot")
                for nh in range(2):
                    po = ps.tile([128, 512], F32, name="po", tag="mm")
                    for d2 in range(2):
                        nc.tensor.matmul(
                            po[:],
                            fT[:, d2, qc * QC:(qc + 1) * QC],
                            wo[:, d2, nh * 512:(nh + 1) * 512],
                            start=(d2 == 0), stop=(d2 == 1))
                    nc.vector.tensor_copy(ot[:, nh * 512:(nh + 1) * 512], po[:])
                nc.sync.dma_start(
                    outp_d.ap()[qc * QC:(qc + 1) * QC, :], ot[:])

            import os as _os
            _order = _os.environ.get("KORDER", "jbd")
            if _order == "seq":
                for h in range(HG):
                    for j in range(NJB):
                        phase_b(h, j)
                for h in range(HG):
                    for qc in range(NQC):
                        phase_c(h, qc)
                for qc in range(NQC):
                    phase_d(qc)
            elif _order == "jb":
                for j in range(NJB):
                    for h in range(HG):
                        phase_b(h, j)
                for h in range(HG):
                    for qc in range(NQC):
                        phase_c(h, qc)
                for qc in range(NQC):
                    phase_d(qc)
            elif _order == "jbd":
                for j in range(NJB):
                    for h in range(HG):
                        phase_b(h, j)
                    for qc in range(4 * j, 4 * j + 4):
                        phase_d(qc)
                for h in range(HG):
                    for qc in range(NQC):
                        phase_c(h, qc)
            elif _order == "hseq":
                for h in range(HG):
                    for j in range(NJB):
                        phase_b(h, j)
                    for qc in range(NQC):
                        phase_c(h, qc)
                for qc in range(NQC):
                    phase_d(qc)
            else:
                for j in range(NJB):
                    for h in range(HG):
                        phase_b(h, j)
                    for h in range(HG):
                        for qc in range(4 * j, 4 * j + 4):
                            phase_c(h, qc)
                    for qc in range(4 * j, 4 * j + 4):
                        phase_d(qc)

    nc.compile()
    return nc


def kernel(qry, key, val, attn_mask, Wq, Wk, Wv, Wo):
    global LAST_EXEC_NS, LAST_RESULTS, bias_idx_n, bias_idx_t

    qry = np.asarray(qry, np.float32)
    key = np.asarray(key, np.float32)
    val = np.asarray(val, np.float32)
    mask = np.asarray(attn_mask).astype(bool)
    Wq, Wk, Wv, Wo = (np.asarray(w, np.float32) for w in (Wq, Wk, Wv, Wo))

    nat_cls, tct_cls = _classify(mask)

    # bias tiles for mixed tiles (shared across batches via the union
    # classification; per-core data still uses the core's own batch mask).
    # Deduped by content (consistent across both batches): a causal mask
    # has only 4 distinct diagonal-crossing patterns per layout.
    biasf = np.where(mask, np.float32(0), np.float32(NEG))  # [B, S, S]
    biasfT = np.ascontiguousarray(biasf.swapaxes(1, 2))

    bias_idx_n, bias_idx_t = {}, {}
    rep_n, rep_t = [], []  # representative (qc,kb)/(kt,j) per unique index
    uniq_n, uniq_t = {}, {}
    for qc in range(NQC):
        for kb in range(NKB):
            if nat_cls[qc, kb] == MIXED:
                t0 = biasf[0, qc * QC:(qc + 1) * QC, kb * KBLK:(kb + 1) * KBLK]
                t1 = biasf[1, qc * QC:(qc + 1) * QC, kb * KBLK:(kb + 1) * KBLK]
                k = (t0.tobytes(), t1.tobytes())
                if k not in uniq_n:
                    uniq_n[k] = len(uniq_n)
                    rep_n.append((qc, kb))
                bias_idx_n[(qc, kb)] = uniq_n[k]
    for kt in range(NKT):
        for j in range(NJB):
            if tct_cls[kt, j] == MIXED:
                t0 = biasfT[0, kt * KT:(kt + 1) * KT, j * JB:(j + 1) * JB]
                t1 = biasfT[1, kt * KT:(kt + 1) * KT, j * JB:(j + 1) * JB]
                k = (t0.tobytes(), t1.tobytes())
                if k not in uniq_t:
                    uniq_t[k] = len(uniq_t)
                    rep_t.append((kt, j))
                bias_idx_t[(kt, j)] = uniq_t[k]
    n_bn, n_bt = len(rep_n), len(rep_t)
    bias_resident = (n_bn + n_bt) <= 16

    # exact active column extent per natural sq-chunk (union over batches)
    anycol = mask.any(axis=0)  # [S, S]
    ext_n = []
    for qc in range(NQC):
        rows = anycol[qc * QC:(qc + 1) * QC]
        nz = np.flatnonzero(rows.any(axis=0))
        ext_n.append(int(nz[-1]) + 1 if nz.size else 0)
    # active sq-column range per transposed mixed tile
    anyT = anycol.T
    rng_t = {}
    for (kt, j) in bias_idx_t:
        tl = anyT[kt * KT:(kt + 1) * KT, j * JB:(j + 1) * JB]
        nz = np.flatnonzero(tl.any(axis=0))
        if nz.size:
            rng_t[(kt, j)] = (int(nz[0]), int(nz[-1]) + 1)
        else:
            rng_t[(kt, j)] = (0, JB)

    cache_key = (nat_cls.tobytes(), tct_cls.tobytes(), bias_resident,
                 tuple(ext_n), tuple(sorted(rng_t.items())))
    if cache_key not in _prog_cache:
        _prog_cache[cache_key] = _build(nat_cls, tct_cls, n_bn, n_bt,
                                        bias_resident, ext_n, rng_t)
    nc = _prog_cache[cache_key]

    # per-batch host prep
    in_maps = []
    for c in range(NCORES):
        b, g = c // 4, c % 4
        cs = slice(DC * g, DC * g + DC)
        m = {
            "ident": np.eye(128, dtype=np.float32),
            "qryT": np.ascontiguousarray(qry[b].T),
            "keyT": np.ascontiguousarray(key[b].T),
            "valT": np.ascontiguousarray(val[b].T),
            "wqT": np.ascontiguousarray(Wq.T[:, cs]),
            "wkT": np.ascontiguousarray(Wk.T[:, cs]),
            "wvT": np.ascontiguousarray(Wv.T[:, cs]),
            "woT": np.ascontiguousarray(Wo.T[cs, :]),
        }
        if n_bn:
            bn = np.empty((n_bn, QC, KBLK), np.float32)
            for i, (qc, kb) in enumerate(rep_n):
                bn[i] = biasf[b, qc * QC:(qc + 1) * QC,
                              kb * KBLK:(kb + 1) * KBLK]
            m["biasN"] = bn
        if n_bt:
            bt = np.empty((n_bt, KT, JB), np.float32)
            for i, (kt, j) in enumerate(rep_t):
                bt[i] = biasfT[b, kt * KT:(kt + 1) * KT, j * JB:(j + 1) * JB]
            m["biasT"] = bt
        in_maps.append(m)

    res = run_bass_kernel_spmd(nc, in_maps, list(range(NCORES)))
    LAST_EXEC_NS = res.exec_time_ns
    LAST_RESULTS = res
    globals()["last_nc"] = nc
    globals()["last_in_maps"] = in_maps

    attn_weights = np.empty((B, H, S, S), np.float32)
    output64 = np.zeros((B, S, D), np.float64)
    for c in range(NCORES):
        b, g = c // 4, c % 4
        attn_weights[b, HG * g:HG * g + HG] = res.results[c]["attn"]
        output64[b] += res.results[c]["outp"]
    output = output64.astype(np.float32)
    return output, attn_weights


# revision 48
# speedup vs baseline: 329.4323x; 1.0249x over previous
"""Multi-head attention (B=2, S=2048, D=1024, H=16) on 8 Trainium2 cores.

Sharding: core c handles batch b=c//4 and head group g=c%4 (4 heads,
d_model slice of 256). Q/K/V/O projections are tensor-parallel over the
head dimension; attention is fully local per core; the output projection
produces per-core partial sums over d_model that the host reduces
(row-parallel W_o, "all-reduce" done in the unshard step).

Returns (output [B,S,D] fp32, attn_weights [B,H,S,S] fp32), matching the
reference module.
"""

import os
import numpy as np

import concourse.bass as bass
import concourse.bacc as bacc
import concourse.mybir as mybir
import concourse.tile as tile
from concourse.bass_utils import run_bass_kernel_spmd

F32 = mybir.dt.float32
F32R = mybir.dt.float32r
AF = mybir.ActivationFunctionType
AX = mybir.AxisListType

B, S, D, H = 2, 2048, 1024, 16
HG = 4              # heads per core
DC = 256            # d_model slice per core (HG * 64)
HD = 64             # head dim
NCORES = 8
SCALE = 0.125       # 1/sqrt(head_dim)
NEG = -1.0e30

QC = 128            # natural-layout sq chunk (psum partitions)
KBLK = 512          # natural-layout sk block (psum free dim)
KT = 128            # transposed-layout sk tile (psum partitions)
JB = 512            # transposed-layout sq block (psum free dim)

NQC, NKB = S // QC, S // KBLK     # 16, 4
NKT, NJB = S // KT, S // JB       # 16, 4

SKIP, FULL, MIXED = 0, 1, 2

LAST_EXEC_NS = None
LAST_RESULTS = None

_prog_cache = {}


def _classify(mask):
    """Per-tile classification over BOTH batches (union-active /
    intersection-full), for the natural [QC x KBLK] and transposed
    [KT x JB] grids. mask: [B, S, S] bool."""
    m = mask.reshape(B, NQC, QC, NKB, KBLK)
    any_n = m.any(axis=(0, 2, 4))          # [NQC, NKB]
    all_n = m.all(axis=(0, 2, 4))
    nat = np.where(all_n, FULL, np.where(any_n, MIXED, SKIP)).astype(np.int32)

    mt = mask.swapaxes(1, 2).reshape(B, NKT, KT, NJB, JB)
    any_t = mt.any(axis=(0, 2, 4))         # [NKT, NJB]
    all_t = mt.all(axis=(0, 2, 4))
    tct = np.where(all_t, FULL, np.where(any_t, MIXED, SKIP)).astype(np.int32)
    return nat, tct


def _build(nat_cls, tct_cls, n_bias_n, n_bias_t, bias_resident, ext_n, rng_t):
    """Build + compile the per-core program. All cores run the same
    program; per-core data differences come through the input tensors."""
    nc = bacc.Bacc("TRN2", target_bir_lowering=False, debug=False,
                   num_devices=NCORES)

    qryT_d = nc.dram_tensor("qryT", [D, S], F32R, kind="ExternalInput")
    keyT_d = nc.dram_tensor("keyT", [D, S], F32R, kind="ExternalInput")
    valT_d = nc.dram_tensor("valT", [D, S], F32R, kind="ExternalInput")
    wqT_d = nc.dram_tensor("wqT", [D, DC], F32R, kind="ExternalInput")
    wkT_d = nc.dram_tensor("wkT", [D, DC], F32R, kind="ExternalInput")
    wvT_d = nc.dram_tensor("wvT", [D, DC], F32R, kind="ExternalInput")
    woT_d = nc.dram_tensor("woT", [DC, D], F32R, kind="ExternalInput")
    bn_d = bt_d = None
    if n_bias_n:
        bn_d = nc.dram_tensor("biasN", [n_bias_n, QC, KBLK], F32R,
                              kind="ExternalInput")
    if n_bias_t:
        bt_d = nc.dram_tensor("biasT", [n_bias_t, KT, JB], F32R,
                              kind="ExternalInput")
    ident_d = nc.dram_tensor("ident", [128, 128], F32R, kind="ExternalInput")

    attn_d = nc.dram_tensor("attn", [HG, S, S], F32, kind="ExternalOutput")
    outp_d = nc.dram_tensor("outp", [S, D], F32, kind="ExternalOutput")
    denr_d = nc.dram_tensor("denr", [HG * NJB, JB], F32)  # internal scratch

    r = F32R

    with tile.TileContext(nc) as tc:
        with (
            tc.tile_pool(name="res", bufs=1) as res,
            tc.tile_pool(name="inq", bufs=3) as inq,
            tc.tile_pool(name="inv", bufs=9) as inv,
            tc.tile_pool(name="ext", bufs=6) as extp,
            tc.tile_pool(name="arow", bufs=5) as arow,
            tc.tile_pool(name="btl", bufs=2) as btl,
            tc.tile_pool(name="sml", bufs=2) as sml,
            tc.tile_pool(name="osb", bufs=2) as osb,
            tc.tile_pool(name="ps", bufs=2, space="PSUM") as ps,
        ):
            # ---- resident SBUF tensors ----
            qT = res.tile([128, 2, S], F32R)     # q^T: d-slice rows, s cols
            kT = res.tile([128, 2, S], F32R)
            fT = res.tile([128, 2, S], F32R)     # normalized feats^T
            v1s = res.tile([128, NKT, HG, HD + 1], F32R)  # [v | ones] per sk-chunk
            wq = res.tile([128, 8, DC], F32R)
            wk = res.tile([128, 8, DC], F32R)
            wv = res.tile([128, 8, DC], F32R)
            wo = res.tile([128, 2, D], F32R)
            ident = res.tile([128, 128], F32R)
            nc.sync.dma_start(ident[:], ident_d.ap())
            if bias_resident:
                bnres = res.tile([128, max(n_bias_n, 1), KBLK], F32R)
                btres = res.tile([128, max(n_bias_t, 1), JB], F32R)
            # fp32r tiles cannot be memset directly; write the denominator
            # ones-column via a rounding copy from an fp32 ones tile
            ones_c = res.tile([128, NKT * HG], F32)
            nc.vector.memset(ones_c[:], 1.0)
            nc.vector.tensor_copy(
                v1s[:, :, :, HD],
                ones_c[:].rearrange("p (a b) -> p a b", b=HG))

            nc.sync.dma_start(wq[:], wqT_d.ap().rearrange("(c p) n -> p c n", p=128))
            nc.sync.dma_start(wk[:], wkT_d.ap().rearrange("(c p) n -> p c n", p=128))
            nc.sync.dma_start(wv[:], wvT_d.ap().rearrange("(c p) n -> p c n", p=128))
            nc.sync.dma_start(wo[:], woT_d.ap().rearrange("(c p) n -> p c n", p=128))
            if bias_resident:
                if n_bias_n:
                    nc.sync.dma_start(
                        bnres[:, 0:n_bias_n, :],
                        bn_d.ap().rearrange("t p n -> p t n"))
                if n_bias_t:
                    nc.sync.dma_start(
                        btres[:, 0:n_bias_t, :],
                        bt_d.ap().rearrange("t p n -> p t n"))

            # ---- Phase A: projections, interleaved by s-block so phase B
            # can start after the first block ----
            for sb in range(S // 512):
                # q^T / k^T: [d_c, S] = (W.T slice).T @ x^T, over D chunks
                for (w_s, x_d, dst, tag) in ((wq, qryT_d, qT, "nat"),
                                             (wk, keyT_d, kT, "mm")):
                    psums = []
                    for hp in range(2):
                        p = ps.tile([128, 512], F32, name=f"pp{hp}", tag=tag)
                        psums.append(p)
                    for dc in range(8):
                        xin = inq.tile([128, 512], F32R, name="xin")
                        nc.sync.dma_start(
                            xin[:],
                            x_d.ap()[dc * 128:(dc + 1) * 128,
                                     sb * 512:(sb + 1) * 512])
                        for hp in range(2):
                            nc.tensor.matmul(
                                psums[hp][:],
                                w_s[:, dc, hp * 128:(hp + 1) * 128],
                                xin[:],
                                start=(dc == 0), stop=(dc == 7))
                    for hp in range(2):
                        nc.vector.tensor_copy(
                            dst[:, hp, sb * 512:(sb + 1) * 512], psums[hp][:])

                # v natural layout: [s, d_c] = (val^T tile).T @ (W_v.T slice)
                vins = []
                for dc in range(8):
                    vin = inv.tile([128, 512], F32R, name="vin")
                    nc.sync.dma_start(
                        vin[:],
                        valT_d.ap()[dc * 128:(dc + 1) * 128,
                                    sb * 512:(sb + 1) * 512])
                    vins.append(vin)
                for s4 in range(4):
                    sc = sb * 4 + s4
                    pv = ps.tile([128, DC], F32, name="pv", tag="ft")
                    for dc in range(8):
                        nc.tensor.matmul(
                            pv[:], vins[dc][:, s4 * 128:(s4 + 1) * 128],
                            wv[:, dc, :],
                            start=(dc == 0), stop=(dc == 7))
                    nc.vector.tensor_copy(
                        v1s[:, sc, :, 0:HD],
                        pv[:].rearrange("p (h d) -> p h d", h=HG))

            # ---- Phases B/C/D interleaved per sq-block j ----
            # B: transposed scores -> exp -> feats^T (+denom)
            # C: natural scores -> exp(+accum) -> normalize -> attn DMA
            # D: partial output projection (needs all heads' feats for j)
            def phase_b(h, j):
                    hp, poff = h // 2, (h % 2) * HD
                    acts = [kt for kt in range(NKT) if tct_cls[kt, j] != SKIP]
                    fsl = fT[poff:poff + HD, hp, j * JB:(j + 1) * JB]
                    if not acts:
                        nc.vector.memset(fsl, 0.0)
                        return
                    pf = ps.tile([HD + 1, JB], F32, name="pf", tag="ft")
                    qsl = qT[poff:poff + HD, hp, j * JB:(j + 1) * JB]
                    nacts = len(acts)
                    for i, kt in enumerate(acts):
                        mixed = tct_cls[kt, j] == MIXED
                        c0, c1 = 0, JB
                        if mixed and i > 0:
                            c0, c1 = rng_t[(kt, j)]
                        w = c1 - c0
                        pst = ps.tile([128, JB], F32, name="pst", tag="mm")
                        nc.tensor.matmul(
                            pst[:, c0:c1],
                            kT[poff:poff + HD, hp,
                               kt * KT:(kt + 1) * KT],
                            qsl[:, c0:c1], start=True, stop=not mixed)
                        if mixed:
                            bi = bias_idx_t[(kt, j)]
                            if bias_resident:
                                nc.tensor.matmul(pst[:, c0:c1], ident[:],
                                                 btres[:, bi, c0:c1],
                                                 start=False, stop=True)
                            else:
                                bb = btl.tile([128, JB], F32R, name="btt")
                                nc.gpsimd.dma_start(bb[:], bt_d.ap()[bi])
                                nc.tensor.matmul(pst[:, c0:c1], ident[:],
                                                 bb[:, c0:c1],
                                                 start=False, stop=True)
                        ext = extp.tile([128, JB], F32R, name="ext")
                        nc.scalar.activation(ext[:, c0:c1], pst[:, c0:c1],
                                             AF.Exp, scale=SCALE)
                        nc.tensor.matmul(
                            pf[:, c0:c1], v1s[:, kt, h, :], ext[:, c0:c1],
                            start=(i == 0), stop=(i == nacts - 1))
                    # denominator row -> reciprocal -> PE ones-broadcast
                    rr = sml.tile([1, JB], F32, name="rr")
                    nc.vector.reciprocal(rr[:], pf[HD:HD + 1, :])
                    scr = denr_d.ap()[h * NJB + j:h * NJB + j + 1, :]
                    nc.gpsimd.dma_start(scr, rr[:])
                    rbc = sml.tile([HD, JB], F32, name="rbc")
                    nc.gpsimd.dma_start(
                        rbc[:],
                        bass.AP(tensor=scr.tensor, offset=scr.offset,
                                ap=[[0, HD]] + scr.ap[1:]))
                    nc.vector.tensor_mul(fsl, pf[0:HD, :], rbc[:])

            def phase_c(h, qc):
                    hp, poff = h // 2, (h % 2) * HD
                    acts = [kb for kb in range(NKB) if nat_cls[qc, kb] != SKIP]
                    if not acts:
                        return  # output buffers are pre-zeroed
                    ncols = ext_n[qc]
                    nend = max(acts) + 1
                    assert nend * KBLK >= ncols > (nend - 1) * KBLK
                    ar = arow.tile([128, S], F32, name="ar")
                    dp = sml.tile([128, NKB], F32, name="dp")
                    # pair consecutive blocks into 2-bank psum tiles
                    kb = 0
                    npart = 0
                    while kb < nend:
                        take = 2 if kb + 1 < nend else 1
                        pn = ps.tile([128, 1024], F32, name="pn", tag="nat")
                        width = 0
                        for t in range(take):
                            blk = kb + t
                            if nat_cls[qc, blk] == SKIP:
                                w = min((blk + 1) * KBLK, ncols) - blk * KBLK
                                if w > 0:
                                    nc.vector.memset(
                                        ar[:, blk * KBLK:blk * KBLK + w], 0.0)
                                continue
                            psl = pn[:, t * KBLK:(t + 1) * KBLK]
                            mixed = nat_cls[qc, blk] == MIXED
                            nc.tensor.matmul(
                                psl,
                                qT[poff:poff + HD, hp,
                                   qc * QC:(qc + 1) * QC],
                                kT[poff:poff + HD, hp,
                                   blk * KBLK:(blk + 1) * KBLK],
                                start=True, stop=not mixed)
                            if mixed:
                                bi = bias_idx_n[(qc, blk)]
                                if bias_resident:
                                    nc.tensor.matmul(psl, ident[:],
                                                     bnres[:, bi, :],
                                                     start=False, stop=True)
                                else:
                                    bb = btl.tile([128, KBLK], F32R, name="btn")
                                    nc.gpsimd.dma_start(bb[:], bn_d.ap()[bi])
                                    nc.tensor.matmul(psl, ident[:], bb[:],
                                                     start=False, stop=True)
                            width += 1
                        if width == 2:
                            w = min((kb + 2) * KBLK, ncols) - kb * KBLK
                            nc.scalar.activation(
                                ar[:, kb * KBLK:kb * KBLK + w], pn[:, 0:w],
                                AF.Exp, scale=SCALE,
                                accum_out=dp[:, npart:npart + 1])
                            npart += 1
                        elif width == 1:
                            blk = kb if nat_cls[qc, kb] != SKIP else kb + 1
                            w = min((blk + 1) * KBLK, ncols) - blk * KBLK
                            nc.scalar.activation(
                                ar[:, blk * KBLK:blk * KBLK + w],
                                pn[:, (blk - kb) * KBLK:(blk - kb) * KBLK + w],
                                AF.Exp, scale=SCALE,
                                accum_out=dp[:, npart:npart + 1])
                            npart += 1
                        kb += take
                    dsum = sml.tile([128, 1], F32, name="dsum")
                    if npart > 1:
                        nc.vector.reduce_sum(dsum[:], dp[:, 0:npart], axis=AX.X)
                    else:
                        nc.vector.tensor_copy(dsum[:], dp[:, 0:1])
                    nc.vector.reciprocal(dsum[:], dsum[:])
                    nc.vector.tensor_scalar_mul(
                        ar[:, 0:ncols], ar[:, 0:ncols], dsum[:])
                    nparts = max(1, min(4, ncols // 512))
                    step = ((ncols + nparts - 1) // nparts + 127) & ~127
                    c = 0
                    while c < ncols:
                        ce = min(c + step, ncols)
                        nc.sync.dma_start(
                            attn_d.ap()[h, qc * QC:(qc + 1) * QC, c:ce],
                            ar[:, c:ce])
                        c = ce

            def phase_d(qc):
                ot = osb.tile([128, D], F32, name="ot")
                for nh in range(2):
                    po = ps.tile([128, 512], F32, name="po", tag="ft")
                    for d2 in range(2):
                        nc.tensor.matmul(
                            po[:],
                            fT[:, d2, qc * QC:(qc + 1) * QC],
                            wo[:, d2, nh * 512:(nh + 1) * 512],
                            start=(d2 == 0), stop=(d2 == 1))
                    nc.vector.tensor_copy(ot[:, nh * 512:(nh + 1) * 512], po[:])
                nc.sync.dma_start(
                    outp_d.ap()[qc * QC:(qc + 1) * QC, :], ot[:])

            import os as _os
            _order = _os.environ.get("KORDER", "jbd")
            if _order == "seq":
                for h in range(HG):
                    for j in range(NJB):
                        phase_b(h, j)
                for h in range(HG):
                    for qc in range(NQC):
                        phase_c(h, qc)
                for qc in range(NQC):
                    phase_d(qc)
            elif _order == "jb":
                for j in range(NJB):
                    for h in range(HG):
                        phase_b(h, j)
                for h in range(HG):
                    for qc in range(NQC):
                        phase_c(h, qc)
                for qc in range(NQC):
                    phase_d(qc)
            elif _order == "jbd":
                for j in range(NJB):
                    for h in range(HG):
                        phase_b(h, j)
                    for qc in range(4 * j, 4 * j + 4):
                        phase_d(qc)
                for h in range(HG):
                    for qc in range(NQC):
                        phase_c(h, qc)
            elif _order == "hseq":
                for h in range(HG):
                    for j in range(NJB):
                        phase_b(h, j)
                    for qc in range(NQC):
                        phase_c(h, qc)
                for qc in range(NQC):
                    phase_d(qc)
            else:
                for j in range(NJB):
                    for h in range(HG):
                        phase_b(h, j)
                    for h in range(HG):
                        for qc in range(4 * j, 4 * j + 4):
                            phase_c(h, qc)
                    for qc in range(4 * j, 4 * j + 4):
                        phase_d(qc)

    nc.compile()
    return nc


def kernel(qry, key, val, attn_mask, Wq, Wk, Wv, Wo):
    global LAST_EXEC_NS, LAST_RESULTS, bias_idx_n, bias_idx_t

    qry = np.asarray(qry, np.float32)
    key = np.asarray(key, np.float32)
    val = np.asarray(val, np.float32)
    mask = np.asarray(attn_mask).astype(bool)
    Wq, Wk, Wv, Wo = (np.asarray(w, np.float32) for w in (Wq, Wk, Wv, Wo))

    nat_cls, tct_cls = _classify(mask)

    # bias tiles for mixed tiles (shared across batches via the union
    # classification; per-core data still uses the core's own batch mask).
    # Deduped by content (consistent across both batches): a causal mask
    # has only 4 distinct diagonal-crossing patterns per layout.
    biasf = np.where(mask, np.float32(0), np.float32(NEG))  # [B, S, S]
    biasfT = np.ascontiguousarray(biasf.swapaxes(1, 2))

    bias_idx_n, bias_idx_t = {}, {}
    rep_n, rep_t = [], []  # representative (qc,kb)/(kt,j) per unique index
    uniq_n, uniq_t = {}, {}
    for qc in range(NQC):
        for kb in range(NKB):
            if nat_cls[qc, kb] == MIXED:
                t0 = biasf[0, qc * QC:(qc + 1) * QC, kb * KBLK:(kb + 1) * KBLK]
                t1 = biasf[1, qc * QC:(qc + 1) * QC, kb * KBLK:(kb + 1) * KBLK]
                k = (t0.tobytes(), t1.tobytes())
                if k not in uniq_n:
                    uniq_n[k] = len(uniq_n)
                    rep_n.append((qc, kb))
                bias_idx_n[(qc, kb)] = uniq_n[k]
    for kt in range(NKT):
        for j in range(NJB):
            if tct_cls[kt, j] == MIXED:
                t0 = biasfT[0, kt * KT:(kt + 1) * KT, j * JB:(j + 1) * JB]
                t1 = biasfT[1, kt * KT:(kt + 1) * KT, j * JB:(j + 1) * JB]
                k = (t0.tobytes(), t1.tobytes())
                if k not in uniq_t:
                    uniq_t[k] = len(uniq_t)
                    rep_t.append((kt, j))
                bias_idx_t[(kt, j)] = uniq_t[k]
    n_bn, n_bt = len(rep_n), len(rep_t)
    bias_resident = (n_bn + n_bt) <= 16

    # exact active column extent per natural sq-chunk (union over batches)
    anycol = mask.any(axis=0)  # [S, S]
    ext_n = []
    for qc in range(NQC):
        rows = anycol[qc * QC:(qc + 1) * QC]
        nz = np.flatnonzero(rows.any(axis=0))
        ext_n.append(int(nz[-1]) + 1 if nz.size else 0)
    # active sq-column range per transposed mixed tile
    anyT = anycol.T
    rng_t = {}
    for (kt, j) in bias_idx_t:
        tl = anyT[kt * KT:(kt + 1) * KT, j * JB:(j + 1) * JB]
        nz = np.flatnonzero(tl.any(axis=0))
        if nz.size:
            rng_t[(kt, j)] = (int(nz[0]), int(nz[-1]) + 1)
        else:
            rng_t[(kt, j)] = (0, JB)

    cache_key = (nat_cls.tobytes(), tct_cls.tobytes(), bias_resident,
                 tuple(ext_n), tuple(sorted(rng_t.items())))
    if cache_key not in _prog_cache:
        _prog_cache[cache_key] = _build(nat_cls, tct_cls, n_bn, n_bt,
                                        bias_resident, ext_n, rng_t)
    nc = _prog_cache[cache_key]

    # per-batch host prep
    in_maps = []
    for c in range(NCORES):
        b, g = c // 4, c % 4
        cs = slice(DC * g, DC * g + DC)
        m = {
            "ident": np.eye(128, dtype=np.float32),
            "qryT": np.ascontiguousarray(qry[b].T),
            "keyT": np.ascontiguousarray(key[b].T),
            "valT": np.ascontiguousarray(val[b].T),
            "wqT": np.ascontiguousarray(Wq.T[:, cs]),
            "wkT": np.ascontiguousarray(Wk.T[:, cs]),
            "wvT": np.ascontiguousarray(Wv.T[:, cs]),
            "woT": np.ascontiguousarray(Wo.T[cs, :]),
        }
        if n_bn:
            bn = np.empty((n_bn, QC, KBLK), np.float32)
            for i, (qc, kb) in enumerate(rep_n):
                bn[i] = biasf[b, qc * QC:(qc + 1) * QC,
                              kb * KBLK:(kb + 1) * KBLK]
            m["biasN"] = bn
        if n_bt:
            bt = np.empty((n_bt, KT, JB), np.float32)
            for i, (kt, j) in enumerate(rep_t):
                bt[i] = biasfT[b, kt * KT:(kt + 1) * KT, j * JB:(j + 1) * JB]
            m["biasT"] = bt
        in_maps.append(m)

    res = run_bass_kernel_spmd(nc, in_maps, list(range(NCORES)))
    LAST_EXEC_NS = res.exec_time_ns
    LAST_RESULTS = res
    globals()["last_nc"] = nc
    globals()["last_in_maps"] = in_maps

    attn_weights = np.empty((B, H, S, S), np.float32)
    output64 = np.zeros((B, S, D), np.float64)
    for c in range(NCORES):
        b, g = c // 4, c % 4
        attn_weights[b, HG * g:HG * g + HG] = res.results[c]["attn"]
        output64[b] += res.results[c]["outp"]
    output = output64.astype(np.float32)
    return output, attn_weights


# revision 53
# speedup vs baseline: 331.5967x; 1.0066x over previous
"""Multi-head attention (B=2, S=2048, D=1024, H=16) on 8 Trainium2 cores.

Sharding: core c handles batch b=c//4 and head group g=c%4 (4 heads,
d_model slice of 256). Q/K/V/O projections are tensor-parallel over the
head dimension; attention is fully local per core; the output projection
produces per-core partial sums over d_model that the host reduces
(row-parallel W_o, "all-reduce" done in the unshard step).

Returns (output [B,S,D] fp32, attn_weights [B,H,S,S] fp32), matching the
reference module.
"""

import os
import numpy as np

import concourse.bass as bass
import concourse.bacc as bacc
import concourse.mybir as mybir
import concourse.tile as tile
from concourse.bass_utils import run_bass_kernel_spmd

F32 = mybir.dt.float32
F32R = mybir.dt.float32r
AF = mybir.ActivationFunctionType
AX = mybir.AxisListType

B, S, D, H = 2, 2048, 1024, 16
HG = 4              # heads per core
DC = 256            # d_model slice per core (HG * 64)
HD = 64             # head dim
NCORES = 8
SCALE = 0.125       # 1/sqrt(head_dim)
NEG = -1.0e30

QC = 128            # natural-layout sq chunk (psum partitions)
KBLK = 512          # natural-layout sk block (psum free dim)
KT = 128            # transposed-layout sk tile (psum partitions)
JB = 512            # transposed-layout sq block (psum free dim)

NQC, NKB = S // QC, S // KBLK     # 16, 4
NKT, NJB = S // KT, S // JB       # 16, 4

SKIP, FULL, MIXED = 0, 1, 2

LAST_EXEC_NS = None
LAST_RESULTS = None

_prog_cache = {}


def _classify(mask):
    """Per-tile classification over BOTH batches (union-active /
    intersection-full), for the natural [QC x KBLK] and transposed
    [KT x JB] grids. mask: [B, S, S] bool."""
    m = mask.reshape(B, NQC, QC, NKB, KBLK)
    any_n = m.any(axis=(0, 2, 4))          # [NQC, NKB]
    all_n = m.all(axis=(0, 2, 4))
    nat = np.where(all_n, FULL, np.where(any_n, MIXED, SKIP)).astype(np.int32)

    mt = mask.swapaxes(1, 2).reshape(B, NKT, KT, NJB, JB)
    any_t = mt.any(axis=(0, 2, 4))         # [NKT, NJB]
    all_t = mt.all(axis=(0, 2, 4))
    tct = np.where(all_t, FULL, np.where(any_t, MIXED, SKIP)).astype(np.int32)
    return nat, tct


def _build(nat_cls, tct_cls, n_bias_n, n_bias_t, bias_resident, ext_n, rng_t):
    """Build + compile the per-core program. All cores run the same
    program; per-core data differences come through the input tensors."""
    nc = bacc.Bacc("TRN2", target_bir_lowering=False, debug=False,
                   num_devices=NCORES)

    qryT_d = nc.dram_tensor("qryT", [D, S], F32R, kind="ExternalInput")
    keyT_d = nc.dram_tensor("keyT", [D, S], F32R, kind="ExternalInput")
    valT_d = nc.dram_tensor("valT", [D, S], F32R, kind="ExternalInput")
    wqT_d = nc.dram_tensor("wqT", [D, DC], F32R, kind="ExternalInput")
    wkT_d = nc.dram_tensor("wkT", [D, DC], F32R, kind="ExternalInput")
    wvT_d = nc.dram_tensor("wvT", [D, DC], F32R, kind="ExternalInput")
    woT_d = nc.dram_tensor("woT", [DC, D], F32R, kind="ExternalInput")
    bn_d = bt_d = None
    if n_bias_n:
        bn_d = nc.dram_tensor("biasN", [n_bias_n, QC, KBLK], F32R,
                              kind="ExternalInput")
    if n_bias_t:
        bt_d = nc.dram_tensor("biasT", [n_bias_t, KT, JB], F32R,
                              kind="ExternalInput")
    ident_d = nc.dram_tensor("ident", [128, 128], F32R, kind="ExternalInput")

    attn_d = nc.dram_tensor("attn", [HG, S, S], F32, kind="ExternalOutput")
    outp_d = nc.dram_tensor("outp", [S, D], F32, kind="ExternalOutput")
    denr_d = nc.dram_tensor("denr", [HG * NJB, JB], F32)  # internal scratch

    r = F32R

    with tile.TileContext(nc) as tc:
        with (
            tc.tile_pool(name="res", bufs=1) as res,
            tc.tile_pool(name="inq", bufs=3) as inq,
            tc.tile_pool(name="inv", bufs=9) as inv,
            tc.tile_pool(name="ext", bufs=7) as extp,
            tc.tile_pool(name="arow", bufs=5) as arow,
            tc.tile_pool(name="btl", bufs=2) as btl,
            tc.tile_pool(name="sml", bufs=2) as sml,
            tc.tile_pool(name="osb", bufs=2) as osb,
            tc.tile_pool(name="ps", bufs=2, space="PSUM") as ps,
        ):
            # ---- resident SBUF tensors ----
            qT = res.tile([128, 2, S], F32R)     # q^T: d-slice rows, s cols
            kT = res.tile([128, 2, S], F32R)
            fT = res.tile([128, 2, S], F32R)     # normalized feats^T
            v1s = res.tile([128, NKT, HG, HD + 1], F32R)  # [v | ones] per sk-chunk
            wq = res.tile([128, 8, DC], F32R)
            wk = res.tile([128, 8, DC], F32R)
            wv = res.tile([128, 8, DC], F32R)
            wo = res.tile([128, 2, D], F32R)
            ident = res.tile([128, 128], F32R)
            nc.sync.dma_start(ident[:], ident_d.ap())
            if bias_resident:
                bnres = res.tile([128, max(n_bias_n, 1), KBLK], F32R)
                btres = res.tile([128, max(n_bias_t, 1), JB], F32R)
            # fp32r tiles cannot be memset directly; write the denominator
            # ones-column via a rounding copy from an fp32 ones tile
            ones_c = res.tile([128, NKT * HG], F32)
            nc.vector.memset(ones_c[:], 1.0)
            nc.vector.tensor_copy(
                v1s[:, :, :, HD],
                ones_c[:].rearrange("p (a b) -> p a b", b=HG))

            nc.sync.dma_start(wq[:], wqT_d.ap().rearrange("(c p) n -> p c n", p=128))
            nc.sync.dma_start(wk[:], wkT_d.ap().rearrange("(c p) n -> p c n", p=128))
            nc.sync.dma_start(wv[:], wvT_d.ap().rearrange("(c p) n -> p c n", p=128))
            nc.sync.dma_start(wo[:], woT_d.ap().rearrange("(c p) n -> p c n", p=128))
            if bias_resident:
                if n_bias_n:
                    nc.sync.dma_start(
                        bnres[:, 0:n_bias_n, :],
                        bn_d.ap().rearrange("t p n -> p t n"))
                if n_bias_t:
                    nc.sync.dma_start(
                        btres[:, 0:n_bias_t, :],
                        bt_d.ap().rearrange("t p n -> p t n"))

            # ---- Phase A: projections, interleaved by s-block so phase B
            # can start after the first block ----
            for sb in range(S // 512):
                # q^T / k^T: [d_c, S] = (W.T slice).T @ x^T, over D chunks
                for (w_s, x_d, dst, tag) in ((wq, qryT_d, qT, "nat"),
                                             (wk, keyT_d, kT, "mm")):
                    psums = []
                    for hp in range(2):
                        p = ps.tile([128, 512], F32, name=f"pp{hp}", tag=tag)
                        psums.append(p)
                    for dc in range(8):
                        xin = inq.tile([128, 512], F32R, name="xin")
                        nc.sync.dma_start(
                            xin[:],
                            x_d.ap()[dc * 128:(dc + 1) * 128,
                                     sb * 512:(sb + 1) * 512])
                        for hp in range(2):
                            nc.tensor.matmul(
                                psums[hp][:],
                                w_s[:, dc, hp * 128:(hp + 1) * 128],
                                xin[:],
                                start=(dc == 0), stop=(dc == 7))
                    for hp in range(2):
                        nc.vector.tensor_copy(
                            dst[:, hp, sb * 512:(sb + 1) * 512], psums[hp][:])

                # v natural layout: [s, d_c] = (val^T tile).T @ (W_v.T slice)
                vins = []
                for dc in range(8):
                    vin = inv.tile([128, 512], F32R, name="vin")
                    nc.sync.dma_start(
                        vin[:],
                        valT_d.ap()[dc * 128:(dc + 1) * 128,
                                    sb * 512:(sb + 1) * 512])
                    vins.append(vin)
                for s4 in range(4):
                    sc = sb * 4 + s4
                    pv = ps.tile([128, DC], F32, name="pv", tag="ft")
                    for dc in range(8):
                        nc.tensor.matmul(
                            pv[:], vins[dc][:, s4 * 128:(s4 + 1) * 128],
                            wv[:, dc, :],
                            start=(dc == 0), stop=(dc == 7))
                    nc.vector.tensor_copy(
                        v1s[:, sc, :, 0:HD],
                        pv[:].rearrange("p (h d) -> p h d", h=HG))

            # ---- Phases B/C/D interleaved per sq-block j ----
            # B: transposed scores -> exp -> feats^T (+denom)
            # C: natural scores -> exp(+accum) -> normalize -> attn DMA
            # D: partial output projection (needs all heads' feats for j)
            def phase_b(h, j):
                    hp, poff = h // 2, (h % 2) * HD
                    acts = [kt for kt in range(NKT) if tct_cls[kt, j] != SKIP]
                    fsl = fT[poff:poff + HD, hp, j * JB:(j + 1) * JB]
                    if not acts:
                        nc.vector.memset(fsl, 0.0)
                        return
                    pf = ps.tile([HD + 1, JB], F32, name="pf", tag="ft")
                    qsl = qT[poff:poff + HD, hp, j * JB:(j + 1) * JB]
                    nacts = len(acts)
                    for i, kt in enumerate(acts):
                        mixed = tct_cls[kt, j] == MIXED
                        c0, c1 = 0, JB
                        if mixed and i > 0:
                            c0, c1 = rng_t[(kt, j)]
                        w = c1 - c0
                        pst = ps.tile([128, JB], F32, name="pst", tag="mm")
                        nc.tensor.matmul(
                            pst[:, c0:c1],
                            kT[poff:poff + HD, hp,
                               kt * KT:(kt + 1) * KT],
                            qsl[:, c0:c1], start=True, stop=not mixed)
                        if mixed:
                            bi = bias_idx_t[(kt, j)]
                            if bias_resident:
                                nc.tensor.matmul(pst[:, c0:c1], ident[:],
                                                 btres[:, bi, c0:c1],
                                                 start=False, stop=True)
                            else:
                                bb = btl.tile([128, JB], F32R, name="btt")
                                nc.gpsimd.dma_start(bb[:], bt_d.ap()[bi])
                                nc.tensor.matmul(pst[:, c0:c1], ident[:],
                                                 bb[:, c0:c1],
                                                 start=False, stop=True)
                        ext = extp.tile([128, JB], F32R, name="ext")
                        nc.scalar.activation(ext[:, c0:c1], pst[:, c0:c1],
                                             AF.Exp, scale=SCALE)
                        nc.tensor.matmul(
                            pf[:, c0:c1], v1s[:, kt, h, :], ext[:, c0:c1],
                            start=(i == 0), stop=(i == nacts - 1))
                    # denominator row -> reciprocal -> PE ones-broadcast
                    rr = sml.tile([1, JB], F32, name="rr")
                    nc.vector.reciprocal(rr[:], pf[HD:HD + 1, :])
                    scr = denr_d.ap()[h * NJB + j:h * NJB + j + 1, :]
                    nc.gpsimd.dma_start(scr, rr[:])
                    rbc = sml.tile([HD, JB], F32, name="rbc")
                    nc.gpsimd.dma_start(
                        rbc[:],
                        bass.AP(tensor=scr.tensor, offset=scr.offset,
                                ap=[[0, HD]] + scr.ap[1:]))
                    nc.vector.tensor_mul(fsl, pf[0:HD, :], rbc[:])

            def phase_c(h, qc):
                    hp, poff = h // 2, (h % 2) * HD
                    acts = [kb for kb in range(NKB) if nat_cls[qc, kb] != SKIP]
                    if not acts:
                        return  # output buffers are pre-zeroed
                    ncols = ext_n[qc]
                    nend = max(acts) + 1
                    assert nend * KBLK >= ncols > (nend - 1) * KBLK
                    ar = arow.tile([128, S], F32, name="ar")
                    dp = sml.tile([128, NKB], F32, name="dp")
                    # pair consecutive blocks into 2-bank psum tiles
                    kb = 0
                    npart = 0
                    while kb < nend:
                        take = 2 if kb + 1 < nend else 1
                        pn = ps.tile([128, 1024], F32, name="pn", tag="nat")
                        width = 0
                        for t in range(take):
                            blk = kb + t
                            if nat_cls[qc, blk] == SKIP:
                                w = min((blk + 1) * KBLK, ncols) - blk * KBLK
                                if w > 0:
                                    nc.vector.memset(
                                        ar[:, blk * KBLK:blk * KBLK + w], 0.0)
                                continue
                            psl = pn[:, t * KBLK:(t + 1) * KBLK]
                            mixed = nat_cls[qc, blk] == MIXED
                            nc.tensor.matmul(
                                psl,
                                qT[poff:poff + HD, hp,
                                   qc * QC:(qc + 1) * QC],
                                kT[poff:poff + HD, hp,
                                   blk * KBLK:(blk + 1) * KBLK],
                                start=True, stop=not mixed)
                            if mixed:
                                bi = bias_idx_n[(qc, blk)]
                                if bias_resident:
                                    nc.tensor.matmul(psl, ident[:],
                                                     bnres[:, bi, :],
                                                     start=False, stop=True)
                                else:
                                    bb = btl.tile([128, KBLK], F32R, name="btn")
                                    nc.gpsimd.dma_start(bb[:], bn_d.ap()[bi])
                                    nc.tensor.matmul(psl, ident[:], bb[:],
                                                     start=False, stop=True)
                            width += 1
                        if width == 2:
                            w = min((kb + 2) * KBLK, ncols) - kb * KBLK
                            nc.scalar.activation(
                                ar[:, kb * KBLK:kb * KBLK + w], pn[:, 0:w],
                                AF.Exp, scale=SCALE,
                                accum_out=dp[:, npart:npart + 1])
                            npart += 1
                        elif width == 1:
                            blk = kb if nat_cls[qc, kb] != SKIP else kb + 1
                            w = min((blk + 1) * KBLK, ncols) - blk * KBLK
                            nc.scalar.activation(
                                ar[:, blk * KBLK:blk * KBLK + w],
                                pn[:, (blk - kb) * KBLK:(blk - kb) * KBLK + w],
                                AF.Exp, scale=SCALE,
                                accum_out=dp[:, npart:npart + 1])
                            npart += 1
                        kb += take
                    dsum = sml.tile([128, 1], F32, name="dsum")
                    if npart > 1:
                        nc.vector.reduce_sum(dsum[:], dp[:, 0:npart], axis=AX.X)
                    else:
                        nc.vector.tensor_copy(dsum[:], dp[:, 0:1])
                    nc.vector.reciprocal(dsum[:], dsum[:])
                    nc.vector.tensor_scalar_mul(
                        ar[:, 0:ncols], ar[:, 0:ncols], dsum[:])
                    nparts = max(1, min(4, ncols // 512))
                    step = ((ncols + nparts - 1) // nparts + 127) & ~127
                    c = 0
                    while c < ncols:
                        ce = min(c + step, ncols)
                        nc.sync.dma_start(
                            attn_d.ap()[h, qc * QC:(qc + 1) * QC, c:ce],
                            ar[:, c:ce])
                        c = ce

            def phase_d(qc):
                ot = osb.tile([128, D], F32, name="ot")
                for nh in range(2):
                    po = ps.tile([128, 512], F32, name="po", tag="ft")
                    for d2 in range(2):
                        nc.tensor.matmul(
                            po[:],
                            fT[:, d2, qc * QC:(qc + 1) * QC],
                            wo[:, d2, nh * 512:(nh + 1) * 512],
                            start=(d2 == 0), stop=(d2 == 1))
                    nc.vector.tensor_copy(ot[:, nh * 512:(nh + 1) * 512], po[:])
                nc.sync.dma_start(
                    outp_d.ap()[qc * QC:(qc + 1) * QC, :], ot[:])

            import os as _os
            _order = _os.environ.get("KORDER", "jbd")
            if _order == "seq":
                for h in range(HG):
                    for j in range(NJB):
                        phase_b(h, j)
                for h in range(HG):
                    for qc in range(NQC):
                        phase_c(h, qc)
                for qc in range(NQC):
                    phase_d(qc)
            elif _order == "jb":
                for j in range(NJB):
                    for h in range(HG):
                        phase_b(h, j)
                for h in range(HG):
                    for qc in range(NQC):
                        phase_c(h, qc)
                for qc in range(NQC):
                    phase_d(qc)
            elif _order == "jbd":
                for j in range(NJB):
                    for h in range(HG):
                        phase_b(h, j)
                    for qc in range(4 * j, 4 * j + 4):
                        phase_d(qc)
                for h in range(HG):
                    for qc in range(NQC):
                        phase_c(h, qc)
            elif _order == "hseq":
                for h in range(HG):
                    for j in range(NJB):
                        phase_b(h, j)
                    for qc in range(NQC):
                        phase_c(h, qc)
                for qc in range(NQC):
                    phase_d(qc)
            else:
                for j in range(NJB):
                    for h in range(HG):
                        phase_b(h, j)
                    for h in range(HG):
                        for qc in range(4 * j, 4 * j + 4):
                            phase_c(h, qc)
                    for qc in range(4 * j, 4 * j + 4):
                        phase_d(qc)

    nc.compile()
    return nc


def kernel(qry, key, val, attn_mask, Wq, Wk, Wv, Wo):
    global LAST_EXEC_NS, LAST_RESULTS, bias_idx_n, bias_idx_t

    qry = np.asarray(qry, np.float32)
    key = np.asarray(key, np.float32)
    val = np.asarray(val, np.float32)
    mask = np.asarray(attn_mask).astype(bool)
    Wq, Wk, Wv, Wo = (np.asarray(w, np.float32) for w in (Wq, Wk, Wv, Wo))

    nat_cls, tct_cls = _classify(mask)

    # bias tiles for mixed tiles (shared across batches via the union
    # classification; per-core data still uses the core's own batch mask).
    # Deduped by content (consistent across both batches): a causal mask
    # has only 4 distinct diagonal-crossing patterns per layout.
    biasf = np.where(mask, np.float32(0), np.float32(NEG))  # [B, S, S]
    biasfT = np.ascontiguousarray(biasf.swapaxes(1, 2))

    bias_idx_n, bias_idx_t = {}, {}
    rep_n, rep_t = [], []  # representative (qc,kb)/(kt,j) per unique index
    uniq_n, uniq_t = {}, {}
    for qc in range(NQC):
        for kb in range(NKB):
            if nat_cls[qc, kb] == MIXED:
                t0 = biasf[0, qc * QC:(qc + 1) * QC, kb * KBLK:(kb + 1) * KBLK]
                t1 = biasf[1, qc * QC:(qc + 1) * QC, kb * KBLK:(kb + 1) * KBLK]
                k = (t0.tobytes(), t1.tobytes())
                if k not in uniq_n:
                    uniq_n[k] = len(uniq_n)
                    rep_n.append((qc, kb))
                bias_idx_n[(qc, kb)] = uniq_n[k]
    for kt in range(NKT):
        for j in range(NJB):
            if tct_cls[kt, j] == MIXED:
                t0 = biasfT[0, kt * KT:(kt + 1) * KT, j * JB:(j + 1) * JB]
                t1 = biasfT[1, kt * KT:(kt + 1) * KT, j * JB:(j + 1) * JB]
                k = (t0.tobytes(), t1.tobytes())
                if k not in uniq_t:
                    uniq_t[k] = len(uniq_t)
                    rep_t.append((kt, j))
                bias_idx_t[(kt, j)] = uniq_t[k]
    n_bn, n_bt = len(rep_n), len(rep_t)
    bias_resident = (n_bn + n_bt) <= 16

    # exact active column extent per natural sq-chunk (union over batches)
    anycol = mask.any(axis=0)  # [S, S]
    ext_n = []
    for qc in range(NQC):
        rows = anycol[qc * QC:(qc + 1) * QC]
        nz = np.flatnonzero(rows.any(axis=0))
        ext_n.append(int(nz[-1]) + 1 if nz.size else 0)
    # active sq-column range per transposed mixed tile
    anyT = anycol.T
    rng_t = {}
    for (kt, j) in bias_idx_t:
        tl = anyT[kt * KT:(kt + 1) * KT, j * JB:(j + 1) * JB]
        nz = np.flatnonzero(tl.any(axis=0))
        if nz.size:
            rng_t[(kt, j)] = (int(nz[0]), int(nz[-1]) + 1)
        else:
            rng_t[(kt, j)] = (0, JB)

    cache_key = (nat_cls.tobytes(), tct_cls.tobytes(), bias_resident,
                 tuple(ext_n), tuple(sorted(rng_t.items())))
    if cache_key not in _prog_cache:
        _prog_cache[cache_key] = _build(nat_cls, tct_cls, n_bn, n_bt,
                                        bias_resident, ext_n, rng_t)
    nc = _prog_cache[cache_key]

    # per-batch host prep
    in_maps = []
    for c in range(NCORES):
        b, g = c // 4, c % 4
        cs = slice(DC * g, DC * g + DC)
        m = {
            "ident": np.eye(128, dtype=np.float32),
            "qryT": np.ascontiguousarray(qry[b].T),
            "keyT": np.ascontiguousarray(key[b].T),
            "valT": np.ascontiguousarray(val[b].T),
            "wqT": np.ascontiguousarray(Wq.T[:, cs]),
            "wkT": np.ascontiguousarray(Wk.T[:, cs]),
            "wvT": np.ascontiguousarray(Wv.T[:, cs]),
            "woT": np.ascontiguousarray(Wo.T[cs, :]),
        }
        if n_bn:
            bn = np.empty((n_bn, QC, KBLK), np.float32)
            for i, (qc, kb) in enumerate(rep_n):
                bn[i] = biasf[b, qc * QC:(qc + 1) * QC,
                              kb * KBLK:(kb + 1) * KBLK]
            m["biasN"] = bn
        if n_bt:
            bt = np.empty((n_bt, KT, JB), np.float32)
            for i, (kt, j) in enumerate(rep_t):
                bt[i] = biasfT[b, kt * KT:(kt + 1) * KT, j * JB:(j + 1) * JB]
            m["biasT"] = bt
        in_maps.append(m)

    res = run_bass_kernel_spmd(nc, in_maps, list(range(NCORES)))
    LAST_EXEC_NS = res.exec_time_ns
    LAST_RESULTS = res
    globals()["last_nc"] = nc
    globals()["last_in_maps"] = in_maps

    attn_weights = np.empty((B, H, S, S), np.float32)
    output64 = np.zeros((B, S, D), np.float64)
    for c in range(NCORES):
        b, g = c // 4, c % 4
        attn_weights[b, HG * g:HG * g + HG] = res.results[c]["attn"]
        output64[b] += res.results[c]["outp"]
    output = output64.astype(np.float32)
    return output, attn_weights


# revision 55
# speedup vs baseline: 333.4891x; 1.0057x over previous
"""Multi-head attention (B=2, S=2048, D=1024, H=16) on 8 Trainium2 cores.

Sharding: core c handles batch b=c//4 and head group g=c%4 (4 heads,
d_model slice of 256). Q/K/V/O projections are tensor-parallel over the
head dimension; attention is fully local per core; the output projection
produces per-core partial sums over d_model that the host reduces
(row-parallel W_o, "all-reduce" done in the unshard step).

Returns (output [B,S,D] fp32, attn_weights [B,H,S,S] fp32), matching the
reference module.
"""

import os
import numpy as np

import concourse.bass as bass
import concourse.bacc as bacc
import concourse.mybir as mybir
import concourse.tile as tile
from concourse.bass_utils import run_bass_kernel_spmd

F32 = mybir.dt.float32
F32R = mybir.dt.float32r
AF = mybir.ActivationFunctionType
AX = mybir.AxisListType

B, S, D, H = 2, 2048, 1024, 16
HG = 4              # heads per core
DC = 256            # d_model slice per core (HG * 64)
HD = 64             # head dim
NCORES = 8
SCALE = 0.125       # 1/sqrt(head_dim)
NEG = -1.0e30

QC = 128            # natural-layout sq chunk (psum partitions)
KBLK = 512          # natural-layout sk block (psum free dim)
KT = 128            # transposed-layout sk tile (psum partitions)
JB = 512            # transposed-layout sq block (psum free dim)

NQC, NKB = S // QC, S // KBLK     # 16, 4
NKT, NJB = S // KT, S // JB       # 16, 4

SKIP, FULL, MIXED = 0, 1, 2

LAST_EXEC_NS = None
LAST_RESULTS = None

_prog_cache = {}


def _classify(mask):
    """Per-tile classification over BOTH batches (union-active /
    intersection-full), for the natural [QC x KBLK] and transposed
    [KT x JB] grids. mask: [B, S, S] bool."""
    m = mask.reshape(B, NQC, QC, NKB, KBLK)
    any_n = m.any(axis=(0, 2, 4))          # [NQC, NKB]
    all_n = m.all(axis=(0, 2, 4))
    nat = np.where(all_n, FULL, np.where(any_n, MIXED, SKIP)).astype(np.int32)

    mt = mask.swapaxes(1, 2).reshape(B, NKT, KT, NJB, JB)
    any_t = mt.any(axis=(0, 2, 4))         # [NKT, NJB]
    all_t = mt.all(axis=(0, 2, 4))
    tct = np.where(all_t, FULL, np.where(any_t, MIXED, SKIP)).astype(np.int32)
    return nat, tct


def _build(nat_cls, tct_cls, n_bias_n, n_bias_t, bias_resident, ext_n, rng_t):
    """Build + compile the per-core program. All cores run the same
    program; per-core data differences come through the input tensors."""
    nc = bacc.Bacc("TRN2", target_bir_lowering=False, debug=False,
                   num_devices=NCORES)

    qryT_d = nc.dram_tensor("qryT", [D, S], F32R, kind="ExternalInput")
    keyT_d = nc.dram_tensor("keyT", [D, S], F32R, kind="ExternalInput")
    valT_d = nc.dram_tensor("valT", [D, S], F32R, kind="ExternalInput")
    wqT_d = nc.dram_tensor("wqT", [D, DC], F32R, kind="ExternalInput")
    wkT_d = nc.dram_tensor("wkT", [D, DC], F32R, kind="ExternalInput")
    wvT_d = nc.dram_tensor("wvT", [D, DC], F32R, kind="ExternalInput")
    woT_d = nc.dram_tensor("woT", [DC, D], F32R, kind="ExternalInput")
    bn_d = bt_d = None
    if n_bias_n:
        bn_d = nc.dram_tensor("biasN", [n_bias_n, QC, KBLK], F32R,
                              kind="ExternalInput")
    if n_bias_t:
        bt_d = nc.dram_tensor("biasT", [n_bias_t, KT, JB], F32R,
                              kind="ExternalInput")
    ident_d = nc.dram_tensor("ident", [128, 128], F32R, kind="ExternalInput")

    attn_d = nc.dram_tensor("attn", [HG, S, S], F32, kind="ExternalOutput")
    outp_d = nc.dram_tensor("outp", [S, D], F32, kind="ExternalOutput")
    denr_d = nc.dram_tensor("denr", [HG * NJB, JB], F32)  # internal scratch

    r = F32R

    with tile.TileContext(nc) as tc:
        with (
            tc.tile_pool(name="res", bufs=1) as res,
            tc.tile_pool(name="inq", bufs=3) as inq,
            tc.tile_pool(name="inv", bufs=9) as inv,
            tc.tile_pool(name="ext", bufs=7) as extp,
            tc.tile_pool(name="arow", bufs=5) as arow,
            tc.tile_pool(name="btl", bufs=2) as btl,
            tc.tile_pool(name="sml", bufs=2) as sml,
            tc.tile_pool(name="osb", bufs=2) as osb,
            tc.tile_pool(name="ps", bufs=2, space="PSUM") as ps,
        ):
            # ---- resident SBUF tensors ----
            qT = res.tile([128, 2, S], F32R)     # q^T: d-slice rows, s cols
            kT = res.tile([128, 2, S], F32R)
            fT = res.tile([128, 2, S], F32R)     # normalized feats^T
            v1s = res.tile([128, NKT, HG, HD + 1], F32R)  # [v | ones] per sk-chunk
            wq = res.tile([128, 8, DC], F32R)
            wk = res.tile([128, 8, DC], F32R)
            wv = res.tile([128, 8, DC], F32R)
            wo = res.tile([128, 2, D], F32R)
            ident = res.tile([128, 128], F32R)
            nc.sync.dma_start(ident[:], ident_d.ap())
            if bias_resident:
                bnres = res.tile([128, max(n_bias_n, 1), KBLK], F32R)
                btres = res.tile([128, max(n_bias_t, 1), JB], F32R)
            # fp32r tiles cannot be memset directly; write the denominator
            # ones-column via a rounding copy from an fp32 ones tile
            ones_c = res.tile([128, NKT * HG], F32)
            nc.vector.memset(ones_c[:], 1.0)
            nc.vector.tensor_copy(
                v1s[:, :, :, HD],
                ones_c[:].rearrange("p (a b) -> p a b", b=HG))

            nc.sync.dma_start(wq[:], wqT_d.ap().rearrange("(c p) n -> p c n", p=128))
            nc.sync.dma_start(wk[:], wkT_d.ap().rearrange("(c p) n -> p c n", p=128))
            nc.sync.dma_start(wv[:], wvT_d.ap().rearrange("(c p) n -> p c n", p=128))
            nc.sync.dma_start(wo[:], woT_d.ap().rearrange("(c p) n -> p c n", p=128))
            if bias_resident:
                if n_bias_n:
                    nc.sync.dma_start(
                        bnres[:, 0:n_bias_n, :],
                        bn_d.ap().rearrange("t p n -> p t n"))
                if n_bias_t:
                    nc.sync.dma_start(
                        btres[:, 0:n_bias_t, :],
                        bt_d.ap().rearrange("t p n -> p t n"))

            # ---- Phase A: projections, interleaved by s-block so phase B
            # can start after the first block ----
            for sb in range(S // 512):
                # q^T / k^T: [d_c, S] = (W.T slice).T @ x^T, over D chunks
                for (w_s, x_d, dst, tag) in ((wq, qryT_d, qT, "nat"),
                                             (wk, keyT_d, kT, "mm")):
                    psums = []
                    for hp in range(2):
                        p = ps.tile([128, 512], F32, name=f"pp{hp}", tag=tag)
                        psums.append(p)
                    for dc in range(8):
                        xin = inq.tile([128, 512], F32R, name="xin")
                        nc.sync.dma_start(
                            xin[:],
                            x_d.ap()[dc * 128:(dc + 1) * 128,
                                     sb * 512:(sb + 1) * 512])
                        for hp in range(2):
                            nc.tensor.matmul(
                                psums[hp][:],
                                w_s[:, dc, hp * 128:(hp + 1) * 128],
                                xin[:],
                                start=(dc == 0), stop=(dc == 7))
                    for hp in range(2):
                        nc.vector.tensor_copy(
                            dst[:, hp, sb * 512:(sb + 1) * 512], psums[hp][:])

                # v natural layout: [s, d_c] = (val^T tile).T @ (W_v.T slice)
                vins = []
                for dc in range(8):
                    vin = inv.tile([128, 512], F32R, name="vin")
                    nc.sync.dma_start(
                        vin[:],
                        valT_d.ap()[dc * 128:(dc + 1) * 128,
                                    sb * 512:(sb + 1) * 512])
                    vins.append(vin)
                for s4 in range(4):
                    sc = sb * 4 + s4
                    pv = ps.tile([128, DC], F32, name="pv", tag="ft")
                    for dc in range(8):
                        nc.tensor.matmul(
                            pv[:], vins[dc][:, s4 * 128:(s4 + 1) * 128],
                            wv[:, dc, :],
                            start=(dc == 0), stop=(dc == 7))
                    nc.vector.tensor_copy(
                        v1s[:, sc, :, 0:HD],
                        pv[:].rearrange("p (h d) -> p h d", h=HG))

            # ---- Phases B/C/D interleaved per sq-block j ----
            # B: transposed scores -> exp -> feats^T (+denom)
            # C: natural scores -> exp(+accum) -> normalize -> attn DMA
            # D: partial output projection (needs all heads' feats for j)
            def phase_b(h, j):
                    hp, poff = h // 2, (h % 2) * HD
                    acts = [kt for kt in range(NKT) if tct_cls[kt, j] != SKIP]
                    fsl = fT[poff:poff + HD, hp, j * JB:(j + 1) * JB]
                    if not acts:
                        nc.vector.memset(fsl, 0.0)
                        return
                    pf = ps.tile([HD + 1, JB], F32, name="pf", tag="ft")
                    qsl = qT[poff:poff + HD, hp, j * JB:(j + 1) * JB]
                    nacts = len(acts)
                    for i, kt in enumerate(acts):
                        mixed = tct_cls[kt, j] == MIXED
                        c0, c1 = 0, JB
                        if mixed and i > 0:
                            c0, c1 = rng_t[(kt, j)]
                        w = c1 - c0
                        pst = ps.tile([128, JB], F32, name="pst", tag="mm")
                        nc.tensor.matmul(
                            pst[:, c0:c1],
                            kT[poff:poff + HD, hp,
                               kt * KT:(kt + 1) * KT],
                            qsl[:, c0:c1], start=True, stop=not mixed)
                        if mixed:
                            bi = bias_idx_t[(kt, j)]
                            if bias_resident:
                                nc.tensor.matmul(pst[:, c0:c1], ident[:],
                                                 btres[:, bi, c0:c1],
                                                 start=False, stop=True)
                            else:
                                bb = btl.tile([128, JB], F32R, name="btt")
                                nc.gpsimd.dma_start(bb[:], bt_d.ap()[bi])
                                nc.tensor.matmul(pst[:, c0:c1], ident[:],
                                                 bb[:, c0:c1],
                                                 start=False, stop=True)
                        ext = extp.tile([128, JB], F32R, name="ext")
                        nc.scalar.activation(ext[:, c0:c1], pst[:, c0:c1],
                                             AF.Exp, scale=SCALE)
                        nc.tensor.matmul(
                            pf[:, c0:c1], v1s[:, kt, h, :], ext[:, c0:c1],
                            start=(i == 0), stop=(i == nacts - 1))
                    # denominator row -> reciprocal -> PE ones-broadcast
                    rr = sml.tile([1, JB], F32, name="rr")
                    nc.vector.reciprocal(rr[:], pf[HD:HD + 1, :])
                    scr = denr_d.ap()[h * NJB + j:h * NJB + j + 1, :]
                    nc.gpsimd.dma_start(scr, rr[:])
                    rbc = sml.tile([HD, JB], F32, name="rbc")
                    nc.gpsimd.dma_start(
                        rbc[:],
                        bass.AP(tensor=scr.tensor, offset=scr.offset,
                                ap=[[0, HD]] + scr.ap[1:]))
                    nc.vector.tensor_mul(fsl, pf[0:HD, :], rbc[:])

            def phase_c(h, qc):
                    hp, poff = h // 2, (h % 2) * HD
                    acts = [kb for kb in range(NKB) if nat_cls[qc, kb] != SKIP]
                    if not acts:
                        return  # output buffers are pre-zeroed
                    ncols = ext_n[qc]
                    nend = max(acts) + 1
                    assert nend * KBLK >= ncols > (nend - 1) * KBLK
                    arw = 512 * ((ncols + KBLK - 1) // KBLK)
                    if arw <= 1024:
                        ar = arow.tile([128, 1024], F32, name="ar_s",
                                       tag="ar_s", bufs=4)
                    else:
                        ar = arow.tile([128, S], F32, name="ar_b",
                                       tag="ar_b", bufs=3)
                    dp = sml.tile([128, NKB], F32, name="dp")
                    # pair consecutive blocks into 2-bank psum tiles
                    kb = 0
                    npart = 0
                    while kb < nend:
                        take = 2 if kb + 1 < nend else 1
                        pn = ps.tile([128, 1024], F32, name="pn", tag="nat")
                        width = 0
                        for t in range(take):
                            blk = kb + t
                            if nat_cls[qc, blk] == SKIP:
                                w = min((blk + 1) * KBLK, ncols) - blk * KBLK
                                if w > 0:
                                    nc.vector.memset(
                                        ar[:, blk * KBLK:blk * KBLK + w], 0.0)
                                continue
                            psl = pn[:, t * KBLK:(t + 1) * KBLK]
                            mixed = nat_cls[qc, blk] == MIXED
                            nc.tensor.matmul(
                                psl,
                                qT[poff:poff + HD, hp,
                                   qc * QC:(qc + 1) * QC],
                                kT[poff:poff + HD, hp,
                                   blk * KBLK:(blk + 1) * KBLK],
                                start=True, stop=not mixed)
                            if mixed:
                                bi = bias_idx_n[(qc, blk)]
                                if bias_resident:
                                    nc.tensor.matmul(psl, ident[:],
                                                     bnres[:, bi, :],
                                                     start=False, stop=True)
                                else:
                                    bb = btl.tile([128, KBLK], F32R, name="btn")
                                    nc.gpsimd.dma_start(bb[:], bn_d.ap()[bi])
                                    nc.tensor.matmul(psl, ident[:], bb[:],
                                                     start=False, stop=True)
                            width += 1
                        if width == 2:
                            w = min((kb + 2) * KBLK, ncols) - kb * KBLK
                            nc.scalar.activation(
                                ar[:, kb * KBLK:kb * KBLK + w], pn[:, 0:w],
                                AF.Exp, scale=SCALE,
                                accum_out=dp[:, npart:npart + 1])
                            npart += 1
                        elif width == 1:
                            blk = kb if nat_cls[qc, kb] != SKIP else kb + 1
                            w = min((blk + 1) * KBLK, ncols) - blk * KBLK
                            nc.scalar.activation(
                                ar[:, blk * KBLK:blk * KBLK + w],
                                pn[:, (blk - kb) * KBLK:(blk - kb) * KBLK + w],
                                AF.Exp, scale=SCALE,
                                accum_out=dp[:, npart:npart + 1])
                            npart += 1
                        kb += take
                    dsum = sml.tile([128, 1], F32, name="dsum")
                    if npart > 1:
                        nc.vector.reduce_sum(dsum[:], dp[:, 0:npart], axis=AX.X)
                    else:
                        nc.vector.tensor_copy(dsum[:], dp[:, 0:1])
                    nc.vector.reciprocal(dsum[:], dsum[:])
                    nc.vector.tensor_scalar_mul(
                        ar[:, 0:ncols], ar[:, 0:ncols], dsum[:])
                    nparts = max(1, min(4, ncols // 512))
                    step = ((ncols + nparts - 1) // nparts + 127) & ~127
                    c = 0
                    while c < ncols:
                        ce = min(c + step, ncols)
                        nc.sync.dma_start(
                            attn_d.ap()[h, qc * QC:(qc + 1) * QC, c:ce],
                            ar[:, c:ce])
                        c = ce

            def phase_d(qc):
                ot = osb.tile([128, D], F32, name="ot")
                for nh in range(2):
                    po = ps.tile([128, 512], F32, name="po", tag="ft")
                    for d2 in range(2):
                        nc.tensor.matmul(
                            po[:],
                            fT[:, d2, qc * QC:(qc + 1) * QC],
                            wo[:, d2, nh * 512:(nh + 1) * 512],
                            start=(d2 == 0), stop=(d2 == 1))
                    nc.vector.tensor_copy(ot[:, nh * 512:(nh + 1) * 512], po[:])
                nc.sync.dma_start(
                    outp_d.ap()[qc * QC:(qc + 1) * QC, :], ot[:])

            import os as _os
            _order = _os.environ.get("KORDER", "jbd")
            if _order == "seq":
                for h in range(HG):
                    for j in range(NJB):
                        phase_b(h, j)
                for h in range(HG):
                    for qc in range(NQC):
                        phase_c(h, qc)
                for qc in range(NQC):
                    phase_d(qc)
            elif _order == "jb":
                for j in range(NJB):
                    for h in range(HG):
                        phase_b(h, j)
                for h in range(HG):
                    for qc in range(NQC):
                        phase_c(h, qc)
                for qc in range(NQC):
                    phase_d(qc)
            elif _order == "jbd":
                for j in range(NJB):
                    for h in range(HG):
                        phase_b(h, j)
                    for qc in range(4 * j, 4 * j + 4):
                        phase_d(qc)
                for h in range(HG):
                    for qc in range(NQC):
                        phase_c(h, qc)
            elif _order == "hseq":
                for h in range(HG):
                    for j in range(NJB):
                        phase_b(h, j)
                    for qc in range(NQC):
                        phase_c(h, qc)
                for qc in range(NQC):
                    phase_d(qc)
            else:
                for j in range(NJB):
                    for h in range(HG):
                        phase_b(h, j)
                    for h in range(HG):
                        for qc in range(4 * j, 4 * j + 4):
                            phase_c(h, qc)
                    for qc in range(4 * j, 4 * j + 4):
                        phase_d(qc)

    nc.compile()
    return nc


def kernel(qry, key, val, attn_mask, Wq, Wk, Wv, Wo):
    global LAST_EXEC_NS, LAST_RESULTS, bias_idx_n, bias_idx_t

    qry = np.asarray(qry, np.float32)
    key = np.asarray(key, np.float32)
    val = np.asarray(val, np.float32)
    mask = np.asarray(attn_mask).astype(bool)
    Wq, Wk, Wv, Wo = (np.asarray(w, np.float32) for w in (Wq, Wk, Wv, Wo))

    nat_cls, tct_cls = _classify(mask)

    # bias tiles for mixed tiles (shared across batches via the union
    # classification; per-core data still uses the core's own batch mask).
    # Deduped by content (consistent across both batches): a causal mask
    # has only 4 distinct diagonal-crossing patterns per layout.
    biasf = np.where(mask, np.float32(0), np.float32(NEG))  # [B, S, S]
    biasfT = np.ascontiguousarray(biasf.swapaxes(1, 2))

    bias_idx_n, bias_idx_t = {}, {}
    rep_n, rep_t = [], []  # representative (qc,kb)/(kt,j) per unique index
    uniq_n, uniq_t = {}, {}
    for qc in range(NQC):
        for kb in range(NKB):
            if nat_cls[qc, kb] == MIXED:
                t0 = biasf[0, qc * QC:(qc + 1) * QC, kb * KBLK:(kb + 1) * KBLK]
                t1 = biasf[1, qc * QC:(qc + 1) * QC, kb * KBLK:(kb + 1) * KBLK]
                k = (t0.tobytes(), t1.tobytes())
                if k not in uniq_n:
                    uniq_n[k] = len(uniq_n)
                    rep_n.append((qc, kb))
                bias_idx_n[(qc, kb)] = uniq_n[k]
    for kt in range(NKT):
        for j in range(NJB):
            if tct_cls[kt, j] == MIXED:
                t0 = biasfT[0, kt * KT:(kt + 1) * KT, j * JB:(j + 1) * JB]
                t1 = biasfT[1, kt * KT:(kt + 1) * KT, j * JB:(j + 1) * JB]
                k = (t0.tobytes(), t1.tobytes())
                if k not in uniq_t:
                    uniq_t[k] = len(uniq_t)
                    rep_t.append((kt, j))
                bias_idx_t[(kt, j)] = uniq_t[k]
    n_bn, n_bt = len(rep_n), len(rep_t)
    bias_resident = (n_bn + n_bt) <= 16

    # exact active column extent per natural sq-chunk (union over batches)
    anycol = mask.any(axis=0)  # [S, S]
    ext_n = []
    for qc in range(NQC):
        rows = anycol[qc * QC:(qc + 1) * QC]
        nz = np.flatnonzero(rows.any(axis=0))
        ext_n.append(int(nz[-1]) + 1 if nz.size else 0)
    # active sq-column range per transposed mixed tile
    anyT = anycol.T
    rng_t = {}
    for (kt, j) in bias_idx_t:
        tl = anyT[kt * KT:(kt + 1) * KT, j * JB:(j + 1) * JB]
        nz = np.flatnonzero(tl.any(axis=0))
        if nz.size:
            rng_t[(kt, j)] = (int(nz[0]), int(nz[-1]) + 1)
        else:
            rng_t[(kt, j)] = (0, JB)

    cache_key = (nat_cls.tobytes(), tct_cls.tobytes(), bias_resident,
                 tuple(ext_n), tuple(sorted(rng_t.items())))
    if cache_key not in _prog_cache:
        _prog_cache[cache_key] = _build(nat_cls, tct_cls, n_bn, n_bt,
                                        bias_resident, ext_n, rng_t)
    nc = _prog_cache[cache_key]

    # per-batch host prep
    in_maps = []
    for c in range(NCORES):
        b, g = c // 4, c % 4
        cs = slice(DC * g, DC * g + DC)
        m = {
            "ident": np.eye(128, dtype=np.float32),
            "qryT": np.ascontiguousarray(qry[b].T),
            "keyT": np.ascontiguousarray(key[b].T),
            "valT": np.ascontiguousarray(val[b].T),
            "wqT": np.ascontiguousarray(Wq.T[:, cs]),
            "wkT": np.ascontiguousarray(Wk.T[:, cs]),
            "wvT": np.ascontiguousarray(Wv.T[:, cs]),
            "woT": np.ascontiguousarray(Wo.T[cs, :]),
        }
        if n_bn:
            bn = np.empty((n_bn, QC, KBLK), np.float32)
            for i, (qc, kb) in enumerate(rep_n):
                bn[i] = biasf[b, qc * QC:(qc + 1) * QC,
                              kb * KBLK:(kb + 1) * KBLK]
            m["biasN"] = bn
        if n_bt:
            bt = np.empty((n_bt, KT, JB), np.float32)
            for i, (kt, j) in enumerate(rep_t):
                bt[i] = biasfT[b, kt * KT:(kt + 1) * KT, j * JB:(j + 1) * JB]
            m["biasT"] = bt
        in_maps.append(m)

    res = run_bass_kernel_spmd(nc, in_maps, list(range(NCORES)))
    LAST_EXEC_NS = res.exec_time_ns
    LAST_RESULTS = res
    globals()["last_nc"] = nc
    globals()["last_in_maps"] = in_maps

    attn_weights = np.empty((B, H, S, S), np.float32)
    output64 = np.zeros((B, S, D), np.float64)
    for c in range(NCORES):
        b, g = c // 4, c % 4
        attn_weights[b, HG * g:HG * g + HG] = res.results[c]["attn"]
        output64[b] += res.results[c]["outp"]
    output = output64.astype(np.float32)
    return output, attn_weights
